# revision 3
# baseline (speedup 1.0000x reference)
"""Trainium2 Bass kernel for the nn_Controller problem.

Math background (verified against the reference scans):
  - The push scan collapses: since ut = sigmoid(...) >= 0, the carry u stays
    >= 0 forever, so s_i = min(prev_stg_i, 0) and stg = [min(prev_stg,0), dt].
  - The read scan is a suffix sum: read_i = 1 - sum_{j>i} stg_j, so
    coef_i = min(stg_i, min(1 - S_{i+1}, 0)) and rt = sum_i coef_i * Val_i.
    (coef at the freshly-pushed slot is min(dt,0) = 0, so vt never enters rt.)

Distribution over 8 cores:
  - LSTM/projection/head phase: feature-parallel over H (each core computes a
    H/8 slice of the hidden state for the full batch, weights sliced per core);
    two AllGathers (h0_new, h1_new) between layers.
  - Stack phase (the memory-heavy part): data-parallel over batch B. Each core
    streams its [T1, B/8, M] slice of prev_Val once: every tile is copied to
    the Val output and reduced against coef into rt on the PE.

Layout convention on device: feature-major ("transposed") [feat, batch] so
that batch is the matmul moving dimension and gate biases are per-partition.
The host wrapper prepares transposed inputs and re-transposes tiny outputs.
"""

from contextlib import ExitStack

import numpy as np

import concourse.bass as bass
import concourse.bacc as bacc
import concourse.mybir as mybir
import concourse.tile as tile
from concourse.bass_utils import run_bass_kernel_spmd

F32 = mybir.dt.float32
AF = mybir.ActivationFunctionType
ALU = mybir.AluOpType

# Full-size problem constants
B, T1, M, D, H, L = 256, 2048, 128, 512, 1024, 2
NCORES = 8


def _chunks(total, step=128):
    return [(o, min(step, total - o)) for o in range(0, total, step)]


def build_program(T1, B, M, D, H, ncores, val_bufs=6):
    """Build the SPMD bass program (same program on every core)."""
    assert B % ncores == 0 and H % ncores == 0
    assert T1 % 128 == 0 and D % 128 == 0 and H % 128 == 0 and M <= 128
    BL = B // ncores          # per-core batch slice for the stack phase
    HL = H // ncores          # per-core feature slice of the hidden state
    NTT = T1 // 128           # number of t-tiles in the stack phase
    DG = 4                    # gates per LSTM cell

    nc = bacc.Bacc("TRN2", target_bir_lowering=False, debug=False,
                   enable_asserts=False, num_devices=ncores)

    def din(name, shape):
        return nc.dram_tensor(name, list(shape), F32, kind="ExternalInput")

    def dout(name, shape):
        return nc.dram_tensor(name, list(shape), F32, kind="ExternalOutput")

    # ---- inputs (per-core arrays supplied via in_maps) ----
    xprT = din("xprT", (D + M, B))          # concat(x, prev_read).T, replicated
    WprojT = din("WprojT", (D + M, D))      # Wproj.T, replicated
    W0T = din("W0T", (D + H, DG * HL))      # per-core gate-column slice
    b0 = din("b0", (HL, DG))
    W1T = din("W1T", (2 * H, DG * HL))
    b1 = din("b1", (HL, DG))
    h0T0 = din("h0T0", (H, B))              # h0[0].T, replicated
    h0T1 = din("h0T1", (H, B))
    c0T0 = din("c0T0", (HL, B))             # c0[0].T feature slice
    c0T1 = din("c0T1", (HL, B))
    WheadT = din("WheadT", (H, M + 1 + D))  # [Wv; Wd; Wo].T, replicated
    nmD = (D + 127) // 128
    bhead = din("bhead", (128, 2 + nmD))
    pstg = din("pstg", (T1, BL))            # prev_stg batch slice
    pval = din("pval", (T1, BL, M))         # prev_Val batch slice
    tri128 = din("tri128", (128, 128))      # tri128[j,i] = 1 if j >= i
    triT = din("triT", (NTT + 1, NTT))      # triT[j,k] = 1 if j > k
    allones = din("allones", (128, 128))
    ident = din("ident", (128, 128))

    # ---- outputs (per-core) ----
    otT_o = dout("otT", (D, BL))
    val_o = dout("val", (T1 + 1, BL, M))
    stg_o = dout("stg", (T1 + 1, BL))
    hh0_o = dout("hh0", (HL, B))
    hh1_o = dout("hh1", (HL, B))
    hc0_o = dout("hc0", (HL, B))
    hc1_o = dout("hc1", (HL, B))
    rtT_o = dout("rtT", (M, BL))

    kD = _chunks(D)           # k-tiles over D
    kDM = _chunks(D + M)      # k-tiles over D+M (xpr)
    kH = _chunks(H)           # k-tiles over H
    mD = _chunks(D)           # m-chunks of the projection / ot outputs
    mM = _chunks(M)           # m-chunks of vt

    with tile.TileContext(nc) as tc, ExitStack() as es:
        cpool = es.enter_context(tc.tile_pool(name="consts", bufs=1))
        kpool = es.enter_context(tc.tile_pool(name="acts", bufs=1))
        wpool = es.enter_context(tc.tile_pool(name="wstream", bufs=3))
        gpool = es.enter_context(tc.tile_pool(name="gates", bufs=2))
        spool = es.enter_context(tc.tile_pool(name="small", bufs=2))
        stgpool = es.enter_context(tc.tile_pool(name="stg", bufs=1))
        vpool = es.enter_context(tc.tile_pool(name="val", bufs=val_bufs))
        # PSUM budget (8 banks): pp 2 + gates 4 + rt 2
        pp = es.enter_context(tc.tile_pool(name="ps", bufs=2, space="PSUM"))
        ppg = es.enter_context(tc.tile_pool(name="psg", bufs=1, space="PSUM"))
        ppr = es.enter_context(tc.tile_pool(name="psr", bufs=2, space="PSUM"))
        dpool = es.enter_context(tc.tile_pool(name="dram", bufs=1, space="DRAM"))

        # ---------- constants ----------
        tri128_s = cpool.tile([128, 128], F32)
        nc.scalar.dma_start(tri128_s[:], tri128[:])
        triT_s = cpool.tile([NTT + 1, NTT], F32)
        nc.scalar.dma_start(triT_s[:], triT[:])
        ones_s = cpool.tile([128, 128], F32)
        nc.scalar.dma_start(ones_s[:], allones[:])
        ident_s = cpool.tile([128, 128], F32)
        nc.scalar.dma_start(ident_s[:], ident[:])
        b0_s = cpool.tile([HL, DG], F32)
        nc.scalar.dma_start(b0_s[:], b0[:])
        b1_s = cpool.tile([HL, DG], F32)
        nc.scalar.dma_start(b1_s[:], b1[:])
        bh_s = cpool.tile([128, 2 + nmD], F32)
        nc.scalar.dma_start(bh_s[:], bhead[:])

        # ---------- phase A: x_aug.T = Wproj @ concat(x, prev_read).T ----------
        xpr_s = kpool.tile([128, len(kDM), B], F32)
        for i, (o, sz) in enumerate(kDM):
            nc.scalar.dma_start(xpr_s[:sz, i, :], xprT[o:o + sz, :])
        xa_s = kpool.tile([128, len(mD), B], F32)
        for mi, (mo, msz) in enumerate(mD):
            ps = pp.tile([128, B], F32, tag="ps")
            for i, (o, sz) in enumerate(kDM):
                wt = wpool.tile([128, max(128, DG * HL)], F32, tag="w")
                nc.scalar.dma_start(wt[:sz, :msz], WprojT[o:o + sz, mo:mo + msz])
                nc.tensor.matmul(ps[:msz, :B], wt[:sz, :msz], xpr_s[:sz, i, :],
                                 start=(i == 0), stop=(i == len(kDM) - 1))
            nc.vector.tensor_copy(xa_s[:msz, mi, :], ps[:msz, :B])

        # ---------- LSTM layers ----------
        def lstm_layer(WT, b_s, hprevT_src, c0T_src, nk_x, xsrc):
            """One feature-sliced LSTM cell. Returns (h_new, c_new) [HL, B].
            xsrc(i)/hprevT_src(i) -> (AP [sz, B], sz) for the k-tiles of the
            cell input and the recurrent input. WT rows are 128-strided."""
            gps = [ppg.tile([HL, B], F32, tag=f"g{g}", name=f"g{g}")
                   for g in range(DG)]
            nk = nk_x + len(kH)
            for i in range(nk):
                src, sz = xsrc(i) if i < nk_x else hprevT_src(i - nk_x)
                wt = wpool.tile([128, max(128, DG * HL)], F32, tag="w")
                nc.scalar.dma_start(wt[:sz, :DG * HL],
                                    WT[128 * i:128 * i + sz, :])
                for g in range(DG):
                    nc.tensor.matmul(gps[g][:, :], wt[:sz, g * HL:(g + 1) * HL],
                                     src, start=(i == 0), stop=(i == nk - 1))
            ig = gpool.tile([HL, B], F32, tag="ig")
            fg = gpool.tile([HL, B], F32, tag="fg")
            gg = gpool.tile([HL, B], F32, tag="gg")
            og = gpool.tile([HL, B], F32, tag="og")
            nc.scalar.activation(ig[:], gps[0][:, :], AF.Sigmoid, bias=b_s[:, 0:1])
            nc.scalar.activation(fg[:], gps[1][:, :], AF.Sigmoid, bias=b_s[:, 1:2])
            nc.scalar.activation(gg[:], gps[2][:, :], AF.Tanh, bias=b_s[:, 2:3])
            nc.scalar.activation(og[:], gps[3][:, :], AF.Sigmoid, bias=b_s[:, 3:4])
            c0s = gpool.tile([HL, B], F32, tag="c0l")
            nc.scalar.dma_start(c0s[:], c0T_src[:])
            cn = gpool.tile([HL, B], F32, tag="cn")
            nc.vector.tensor_tensor(cn[:], fg[:], c0s[:], ALU.mult)
            nc.vector.tensor_tensor(ig[:], ig[:], gg[:], ALU.mult)
            nc.vector.tensor_tensor(cn[:], cn[:], ig[:], ALU.add)
            tct = gpool.tile([HL, B], F32, tag="tanhc")
            nc.scalar.activation(tct[:], cn[:], AF.Tanh)
            hn = gpool.tile([HL, B], F32, tag="hn")
            nc.vector.tensor_tensor(hn[:], og[:], tct[:], ALU.mult)
            return hn, cn

        def allgather_h(hn, gname, to_sbuf):
            """AllGather [HL, B] slices across cores -> dram [ncores, HL, B];
            optionally reload the full [H, B] into sbuf k-tiles."""
            bounce = dpool.tile([HL, B], F32, tag=f"{gname}_in")
            nc.gpsimd.dma_start(bounce[:], hn[:])
            gout = dpool.tile([ncores, HL, B], F32, tag=f"{gname}_out")
            nc.gpsimd.collective_compute(
                "AllGather", ALU.bypass,
                replica_groups=[list(range(ncores))],
                ins=[bounce[:].opt()], outs=[gout[:].opt()])
            if not to_sbuf:
                return None, gout
            hall = kpool.tile([128, len(kH), B], F32, tag=f"{gname}_all")
            gflat = gout[:].rearrange("c h b -> (c h) b")
            for i, (o, sz) in enumerate(kH):
                nc.scalar.dma_start(hall[:sz, i, :], gflat[o:o + sz, :])
            return hall, gout

        # layer 0: inputs = x_aug (K=D) + h0[0] (K=H)
        h0T0_s = kpool.tile([128, len(kH), B], F32)
        for i, (o, sz) in enumerate(kH):
            nc.scalar.dma_start(h0T0_s[:sz, i, :], h0T0[o:o + sz, :])
        h0n, c0n = lstm_layer(
            W0T, b0_s,
            hprevT_src=lambda i: (h0T0_s[:kH[i][1], i, :], kH[i][1]),
            c0T_src=c0T0, nk_x=len(kD),
            xsrc=lambda i: (xa_s[:kD[i][1], i, :], kD[i][1]))
        nc.gpsimd.dma_start(hh0_o[:], h0n[:])
        nc.gpsimd.dma_start(hc0_o[:], c0n[:])
        h0n_all, _ = allgather_h(h0n, "g0", to_sbuf=True)

        # layer 1: inputs = h0_new (K=H) + h0[1] (K=H)
        h0T1_s = kpool.tile([128, len(kH), B], F32)
        for i, (o, sz) in enumerate(kH):
            nc.scalar.dma_start(h0T1_s[:sz, i, :], h0T1[o:o + sz, :])
        h1n, c1n = lstm_layer(
            W1T, b1_s,
            hprevT_src=lambda i: (h0T1_s[:kH[i][1], i, :], kH[i][1]),
            c0T_src=c0T1, nk_x=len(kH),
            xsrc=lambda i: (h0n_all[:kH[i][1], i, :], kH[i][1]))
        nc.gpsimd.dma_start(hh1_o[:], h1n[:])
        nc.gpsimd.dma_start(hc1_o[:], c1n[:])
        _, g1out = allgather_h(h1n, "g1", to_sbuf=False)

        # ---------- phase D: heads, on this core's own batch slice ----------
        # Slice the gathered h1_new down to our BL batch columns using the
        # runtime core id (the gather is laid out [core, HL, B]).
        rid = nc.gpsimd.partition_id()
        h1loc = kpool.tile([128, len(kH), BL], F32)
        g1v = g1out[:].rearrange("c h (r b) -> (c h) r b", b=BL)
        for i, (o, sz) in enumerate(kH):
            nc.gpsimd.dma_start(h1loc[:sz, i, :],
                                g1v[o:o + sz, bass.ds(rid, 1), :])

        def head(mcols, func, bias_ap, out_sb):
            co, csz = mcols
            ps = pp.tile([128, BL], F32, tag="ps")
            for i, (o, sz) in enumerate(kH):
                wt = wpool.tile([128, 128], F32, tag="wh")
                nc.scalar.dma_start(wt[:sz, :csz], WheadT[o:o + sz, co:co + csz])
                nc.tensor.matmul(ps[:csz, :BL], wt[:sz, :csz], h1loc[:sz, i, :],
                                 start=(i == 0), stop=(i == len(kH) - 1))
            nc.scalar.activation(out_sb, ps[:csz, :BL], func, bias=bias_ap)

        vt_s = spool.tile([M, BL], F32, tag="vt")
        for mo, msz in mM:
            head((mo, msz), AF.Tanh, bh_s[mo:mo + msz, 0:1], vt_s[mo:mo + msz, :])
        d_s = spool.tile([1, BL], F32, tag="d")
        head((M, 1), AF.Sigmoid, bh_s[0:1, 1:2], d_s[:])
        for mi, (mo, msz) in enumerate(mD):
            o_s = spool.tile([128, BL], F32, tag="ot")
            head((M + 1 + mo, msz), AF.Tanh, bh_s[:msz, 2 + mi:3 + mi],
                 o_s[:msz, :])
            nc.gpsimd.dma_start(otT_o[mo:mo + msz, :], o_s[:msz, :])

        # vt also goes (untransposed) into the last Val row
        vtp = pp.tile([BL, 128], F32, tag="ps")
        nc.tensor.transpose(vtp[:, :M], vt_s[:], ident_s[:M, :M])
        vtn = spool.tile([BL, M], F32, tag="vtn")
        nc.vector.tensor_copy(vtn[:], vtp[:, :M])
        nc.gpsimd.dma_start(val_o[T1:T1 + 1, :, :], vtn[:])
        nc.gpsimd.dma_start(stg_o[T1:T1 + 1, :], d_s[:])

        # ---------- phase E: stg clamp, suffix sums, coef ----------
        stgc = stgpool.tile([128, NTT, BL], F32)
        for t in range(NTT):
            raw = spool.tile([128, BL], F32, tag="praw")
            nc.scalar.dma_start(raw[:], pstg[128 * t:128 * (t + 1), :])
            nc.vector.tensor_scalar_min(stgc[:, t, :], raw[:], 0.0)
            nc.scalar.dma_start(stg_o[128 * t:128 * (t + 1), :], stgc[:, t, :])

        # per-tile totals: ones_col.T @ stgc  -> [1, NTT*BL] in one matmul
        tsp = pp.tile([1, NTT * BL], F32, tag="ps")
        nc.tensor.matmul(tsp[:], ones_s[:, 0:1],
                         stgc[:].rearrange("p t b -> p (t b)"))
        tsrow = spool.tile([1, (NTT + 1) * BL], F32, tag="tsrow")
        nc.vector.tensor_copy(tsrow[:, :NTT * BL], tsp[:])
        nc.vector.tensor_copy(tsrow[:, NTT * BL:], d_s[:])
        # partition-scatter via DRAM bounce: [1,(NTT+1)*BL] -> [(NTT+1), BL]
        tb = dpool.tile([NTT + 1, BL], F32, tag="tsb")
        nc.gpsimd.dma_start(tb[:].rearrange("t b -> (t b)")[None, :], tsrow[:])
        ts17 = spool.tile([NTT + 1, BL], F32, tag="ts17")
        nc.gpsimd.dma_start(ts17[:], tb[:])
        # exclusive tail per tile (includes dt): triT.T @ ts17 -> [NTT, BL]
        tailp = pp.tile([NTT, BL], F32, tag="ps")
        nc.tensor.matmul(tailp[:], triT_s[:], ts17[:])
        tail_sb = spool.tile([NTT, BL], F32, tag="tailsb")
        nc.vector.tensor_copy(tail_sb[:], tailp[:])
        tb2 = dpool.tile([NTT, BL], F32, tag="tsb2")
        nc.gpsimd.dma_start(tb2[:], tail_sb[:])
        tailrow = spool.tile([1, NTT * BL], F32, tag="tailrow")
        nc.gpsimd.dma_start(tailrow[:],
                            tb2[:].rearrange("t b -> (t b)")[None, :])

        coef = stgpool.tile([128, NTT, BL], F32)
        for t in range(NTT):
            sf = pp.tile([128, BL], F32, tag="ps")
            nc.tensor.matmul(sf[:], tri128_s[:], stgc[:, t, :],
                             start=True, stop=False)
            nc.tensor.matmul(sf[:], ones_s[0:1, :],
                             tailrow[:, t * BL:(t + 1) * BL],
                             start=False, stop=True)
            tmp = spool.tile([128, BL], F32, tag="ctmp")
            nc.vector.tensor_tensor(tmp[:], stgc[:, t, :], sf[:], ALU.subtract)
            nc.vector.tensor_scalar_add(tmp[:], tmp[:], 1.0)
            nc.vector.tensor_scalar_min(tmp[:], tmp[:], 0.0)
            nc.vector.tensor_tensor(coef[:, t, :], stgc[:, t, :], tmp[:], ALU.min)

        # ---------- phase F: stream prev_Val: copy out + reduce into rt ----------
        rt_acc = spool.tile([M, BL], F32, tag="rtacc")
        nc.vector.memset(rt_acc[:], 0.0)
        pvv = pval[:].rearrange("t b m -> t (b m)")
        vvv = val_o[:].rearrange("t b m -> t (b m)")
        for t in range(NTT):
            vt_t = vpool.tile([128, BL * M], F32, tag="val")
            nc.sync.dma_start(vt_t[:], pvv[128 * t:128 * (t + 1), :])
            nc.sync.dma_start(vvv[128 * t:128 * (t + 1), :], vt_t[:])
            rp = ppr.tile([M, BL], F32, tag="psR")
            for b in range(BL):
                nc.tensor.matmul(rp[:, b:b + 1], vt_t[:, b * M:(b + 1) * M],
                                 coef[:, t, b:b + 1])
            nc.vector.tensor_tensor(rt_acc[:], rt_acc[:], rp[:], ALU.add)
        nc.gpsimd.dma_start(rtT_o[:], rt_acc[:])

    nc.compile()
    return nc


_PROG_CACHE = {}


def _get_program(key, **kw):
    if key not in _PROG_CACHE:
        _PROG_CACHE[key] = build_program(**kw)
    return _PROG_CACHE[key]


def make_in_maps(inputs, T1, B, M, D, H, ncores):
    """Host-side data prep: transposes/slices only (no real computation)."""
    f = np.float32
    g = {k: np.asarray(v, dtype=f) for k, v in inputs.items()}
    BL, HL = B // ncores, H // ncores
    NTT = T1 // 128
    DG = 4

    xprT = np.ascontiguousarray(np.concatenate([g["x"][0], g["prev_read"]], 1).T)
    WprojT = np.ascontiguousarray(g["Wproj"].T)
    W0Tfull = np.concatenate([g["Wih0"], g["Whh0"]], axis=1).T  # [D+H, 4H]
    W1Tfull = np.concatenate([g["Wih1"], g["Whh1"]], axis=1).T  # [2H, 4H]
    b0full = g["bih0"] + g["bhh0"]
    b1full = g["bih1"] + g["bhh1"]
    h0T0 = np.ascontiguousarray(g["h0"][0].T)
    h0T1 = np.ascontiguousarray(g["h0"][1].T)
    WheadT = np.ascontiguousarray(
        np.concatenate([g["Wv"], g["Wd"], g["Wo"]], axis=0).T)  # [H, M+1+D]
    nmD = (D + 127) // 128
    bhead = np.zeros((128, 2 + nmD), f)
    bhead[:M, 0] = g["bv"]
    bhead[0, 1] = g["bd"][0]
    for mi in range(nmD):
        sz = min(128, D - mi * 128)
        bhead[:sz, 2 + mi] = g["bo"][mi * 128:mi * 128 + sz]

    tri128 = np.tril(np.ones((128, 128), f))           # tri128[j,i]=1 iff j>=i
    triT = np.zeros((NTT + 1, NTT), f)                 # triT[j,k]=1 iff j>k
    for k in range(NTT):
        triT[k + 1:, k] = 1.0
    allones = np.ones((128, 128), f)
    ident = np.eye(128, dtype=f)

    in_maps = []
    for r in range(ncores):
        hs = slice(r * HL, (r + 1) * HL)
        bs = slice(r * BL, (r + 1) * BL)
        gate_cols = np.concatenate(
            [np.arange(gg * H + r * HL, gg * H + (r + 1) * HL)
             for gg in range(DG)])
        in_maps.append({
            "xprT": xprT,
            "WprojT": WprojT,
            "W0T": np.ascontiguousarray(W0Tfull[:, gate_cols]),
            "b0": np.ascontiguousarray(
                b0full[gate_cols].reshape(DG, HL).T),
            "W1T": np.ascontiguousarray(W1Tfull[:, gate_cols]),
            "b1": np.ascontiguousarray(
                b1full[gate_cols].reshape(DG, HL).T),
            "h0T0": h0T0, "h0T1": h0T1,
            "c0T0": np.ascontiguousarray(g["c0"][0].T[hs]),
            "c0T1": np.ascontiguousarray(g["c0"][1].T[hs]),
            "WheadT": WheadT, "bhead": bhead,
            "pstg": np.ascontiguousarray(g["prev_stg"][:, bs]),
            "pval": np.ascontiguousarray(g["prev_Val"][:, bs]),
            "tri128": tri128, "triT": triT,
            "allones": allones, "ident": ident,
        })
    return in_maps


def assemble_outputs(results, T1, B, M, D, H, ncores):
    f = np.float32
    BL, HL = B // ncores, H // ncores
    ot = np.empty((1, B, D), f)
    Val = np.empty((T1 + 1, B, M), f)
    stg = np.empty((T1 + 1, B), f)
    hh = np.empty((2, B, H), f)
    hc = np.empty((2, B, H), f)
    rt = np.empty((B, M), f)
    for r in range(ncores):
        hs = slice(r * HL, (r + 1) * HL)
        bs = slice(r * BL, (r + 1) * BL)
        res = results[r]
        ot[0, bs, :] = res["otT"].T
        Val[:, bs, :] = res["val"]
        stg[:, bs] = res["stg"]
        hh[0, :, hs] = res["hh0"].T
        hh[1, :, hs] = res["hh1"].T
        hc[0, :, hs] = res["hc0"].T
        hc[1, :, hs] = res["hc1"].T
        rt[bs, :] = res["rtT"].T
    return (ot, Val, stg, hh, hc, rt)


def kernel(**inputs):
    nc = _get_program("full", T1=T1, B=B, M=M, D=D, H=H, ncores=NCORES)
    in_maps = make_in_maps(inputs, T1, B, M, D, H, NCORES)
    res = run_bass_kernel_spmd(nc, in_maps, list(range(NCORES)))
    return assemble_outputs(res.results, T1, B, M, D, H, NCORES)


# revision 17
# speedup vs baseline: 1.1915x; 1.1915x over previous
"""Trainium2 Bass kernel for the nn_Controller problem.

Math background (verified against the reference scans):
  - The push scan collapses: since ut = sigmoid(...) >= 0, the carry u stays
    >= 0 forever, so s_i = min(prev_stg_i, 0) and stg = [min(prev_stg,0), dt].
  - The read scan is a suffix sum: read_i = 1 - sum_{j>i} stg_j, so
    coef_i = min(stg_i, min(1 - S_{i+1}, 0)) and rt = sum_i coef_i * Val_i.
    (coef at the freshly-pushed slot is min(dt,0) = 0, so vt never enters rt.)

Distribution over 8 cores:
  - LSTM/projection/head phase: feature-parallel over H (each core computes a
    H/8 slice of the hidden state for the full batch, weights sliced per core);
    two AllGathers (h0_new, h1_new) between layers.
  - Stack phase (the memory-heavy part): data-parallel over batch B. Each core
    streams its [T1, B/8, M] slice of prev_Val once: every tile is copied to
    the Val output and reduced against coef into rt on the PE.

Layout convention on device: feature-major ("transposed") [feat, batch] so
that batch is the matmul moving dimension and gate biases are per-partition.
The host wrapper prepares transposed inputs and re-transposes tiny outputs.
"""

from contextlib import ExitStack

import numpy as np

import concourse.bass as bass
import concourse.bacc as bacc
import concourse.mybir as mybir
import concourse.tile as tile
from concourse.bass_utils import run_bass_kernel_spmd

F32 = mybir.dt.float32
F32R = mybir.dt.float32r
AF = mybir.ActivationFunctionType
ALU = mybir.AluOpType


def _r(ap):
    """View an fp32 AP as float32r for fast PE matmuls (1 cyc/row at N>=256)."""
    return ap.bitcast(F32R)

# Full-size problem constants
B, T1, M, D, H, L = 256, 2048, 128, 512, 1024, 2
NCORES = 8


def _chunks(total, step=128):
    return [(o, min(step, total - o)) for o in range(0, total, step)]


def build_program(T1, B, M, D, H, ncores, val_bufs=4):
    """Build the SPMD bass program (same program on every core)."""
    assert B % ncores == 0 and H % ncores == 0
    assert T1 % 128 == 0 and D % 128 == 0 and H % 128 == 0 and M <= 128
    BL = B // ncores          # per-core batch slice for the stack phase
    HL = H // ncores          # per-core feature slice of the hidden state
    NTT = T1 // 128           # number of t-tiles in the stack phase
    DG = 4                    # gates per LSTM cell

    nc = bacc.Bacc("TRN2", target_bir_lowering=False, debug=False,
                   enable_asserts=False, num_devices=ncores)

    def din(name, shape):
        return nc.dram_tensor(name, list(shape), F32, kind="ExternalInput")

    def dout(name, shape):
        return nc.dram_tensor(name, list(shape), F32, kind="ExternalOutput")

    # ---- inputs (per-core arrays supplied via in_maps) ----
    xprT = din("xprT", (D + M, B))          # concat(x, prev_read).T, replicated
    WprojT = din("WprojT", (D + M, D))      # Wproj.T, replicated
    W0T = din("W0T", (D + H, DG * HL))      # per-core gate-column slice
    b0 = din("b0", (HL, DG))
    W1T = din("W1T", (2 * H, DG * HL))
    b1 = din("b1", (HL, DG))
    h0T0 = din("h0T0", (H, B))              # h0[0].T, replicated
    h0T1 = din("h0T1", (H, B))
    c0T0 = din("c0T0", (HL, B))             # c0[0].T feature slice
    c0T1 = din("c0T1", (HL, B))
    WheadT = din("WheadT", (H, M + 1 + D))  # [Wv; Wd; Wo].T, replicated
    nmD = (D + 127) // 128
    bhead = din("bhead", (128, 2 + nmD))
    pstg = din("pstg", (T1, BL))            # prev_stg batch slice
    pval = din("pval", (T1, BL, M))         # prev_Val batch slice
    tri128 = din("tri128", (128, 128))      # tri128[j,i] = 1 if j >= i
    triT = din("triT", (NTT + 1, NTT))      # triT[j,k] = 1 if j > k
    allones = din("allones", (128, 128))
    ident = din("ident", (128, 128))

    # ---- outputs (per-core) ----
    otT_o = dout("otT", (D, BL))
    val_o = dout("val", (T1 + 1, BL, M))
    stg_o = dout("stg", (T1 + 1, BL))
    hh0_o = dout("hh0", (HL, B))
    hh1_o = dout("hh1", (HL, B))
    hc0_o = dout("hc0", (HL, B))
    hc1_o = dout("hc1", (HL, B))
    rt_o = dout("rt", (BL, M))

    kD = _chunks(D)           # k-tiles over D
    kDM = _chunks(D + M)      # k-tiles over D+M (xpr)
    kH = _chunks(H)           # k-tiles over H
    mD = _chunks(D)           # m-chunks of the projection / ot outputs
    mM = _chunks(M)           # m-chunks of vt

    with tile.TileContext(nc) as tc, ExitStack() as es:
        cpool = es.enter_context(tc.tile_pool(name="consts", bufs=1))
        kpool = es.enter_context(tc.tile_pool(name="acts", bufs=1))
        wpool = es.enter_context(tc.tile_pool(name="wstream", bufs=3))
        gpool = es.enter_context(tc.tile_pool(name="gates", bufs=2))
        spool = es.enter_context(tc.tile_pool(name="small", bufs=2))
        stgpool = es.enter_context(tc.tile_pool(name="stg", bufs=1))
        vpool = es.enter_context(tc.tile_pool(name="val", bufs=val_bufs))
        # PSUM budget (8 banks): pp 2 + gates 4
        pp = es.enter_context(tc.tile_pool(name="ps", bufs=2, space="PSUM"))
        ppg = es.enter_context(tc.tile_pool(name="psg", bufs=1, space="PSUM"))
        dpool = es.enter_context(tc.tile_pool(name="dram", bufs=1, space="DRAM"))

        # ---------- constants ----------
        tri128_s = cpool.tile([128, 128], F32)
        nc.scalar.dma_start(tri128_s[:], tri128[:])
        triT_s = cpool.tile([NTT + 1, NTT], F32)
        nc.scalar.dma_start(triT_s[:], triT[:])
        ones_s = cpool.tile([128, 128], F32)
        nc.scalar.dma_start(ones_s[:], allones[:])
        ident_s = cpool.tile([128, 128], F32)
        nc.scalar.dma_start(ident_s[:], ident[:])
        ones_r = cpool.tile([128, 1], F32)
        nc.scalar.dma_start(_r(ones_r[:]), _r(allones[:, 0:1]))
        b0_s = cpool.tile([HL, DG], F32)
        nc.scalar.dma_start(b0_s[:], b0[:])
        b1_s = cpool.tile([HL, DG], F32)
        nc.scalar.dma_start(b1_s[:], b1[:])
        bh_s = cpool.tile([128, 2 + nmD], F32)
        nc.scalar.dma_start(bh_s[:], bhead[:])

        # ---------- phase A: x_aug.T = Wproj @ concat(x, prev_read).T ----------
        xpr_s = kpool.tile([128, len(kDM), B], F32)
        for i, (o, sz) in enumerate(kDM):
            nc.scalar.dma_start(_r(xpr_s[:sz, i, :]), _r(xprT[o:o + sz, :]))
        xa_s = kpool.tile([128, len(mD), B], F32)
        for mi, (mo, msz) in enumerate(mD):
            ps = pp.tile([128, B], F32, tag="ps")
            for i, (o, sz) in enumerate(kDM):
                wt = wpool.tile([128, max(128, DG * HL)], F32, tag="w")
                nc.scalar.dma_start(_r(wt[:sz, :msz]), _r(WprojT[o:o + sz, mo:mo + msz]))
                nc.tensor.matmul(ps[:msz, :B], _r(wt[:sz, :msz]),
                                 _r(xpr_s[:sz, i, :]),
                                 start=(i == 0), stop=(i == len(kDM) - 1))
            nc.vector.tensor_copy(_r(xa_s[:msz, mi, :]), ps[:msz, :B])

        # ---------- LSTM layers ----------
        def lstm_layer(WT, b_s, hprevT_src, c0T_src, nk_x, xsrc):
            """One feature-sliced LSTM cell. Returns (h_new, c_new) [HL, B].
            xsrc(i)/hprevT_src(i) -> (AP [sz, B], sz) for the k-tiles of the
            cell input and the recurrent input. WT rows are 128-strided."""
            gps = [ppg.tile([HL, B], F32, tag=f"g{g}", name=f"g{g}")
                   for g in range(DG)]
            nk = nk_x + len(kH)
            for i in range(nk):
                src, sz = xsrc(i) if i < nk_x else hprevT_src(i - nk_x)
                wt = wpool.tile([128, max(128, DG * HL)], F32, tag="w")
                nc.scalar.dma_start(_r(wt[:sz, :DG * HL]),
                                    _r(WT[128 * i:128 * i + sz, :]))
                for g in range(DG):
                    nc.tensor.matmul(gps[g][:, :],
                                     _r(wt[:sz, g * HL:(g + 1) * HL]),
                                     _r(src), start=(i == 0), stop=(i == nk - 1))
            ig = gpool.tile([HL, B], F32, tag="ig")
            fg = gpool.tile([HL, B], F32, tag="fg")
            gg = gpool.tile([HL, B], F32, tag="gg")
            og = gpool.tile([HL, B], F32, tag="og")
            nc.scalar.activation(ig[:], gps[0][:, :], AF.Sigmoid, bias=b_s[:, 0:1])
            nc.scalar.activation(fg[:], gps[1][:, :], AF.Sigmoid, bias=b_s[:, 1:2])
            nc.scalar.activation(gg[:], gps[2][:, :], AF.Tanh, bias=b_s[:, 2:3])
            nc.scalar.activation(og[:], gps[3][:, :], AF.Sigmoid, bias=b_s[:, 3:4])
            c0s = gpool.tile([HL, B], F32, tag="c0l")
            nc.scalar.dma_start(c0s[:], c0T_src[:])
            cn = gpool.tile([HL, B], F32, tag="cn")
            nc.vector.tensor_tensor(cn[:], fg[:], c0s[:], ALU.mult)
            nc.vector.tensor_tensor(ig[:], ig[:], gg[:], ALU.mult)
            nc.vector.tensor_tensor(cn[:], cn[:], ig[:], ALU.add)
            tct = gpool.tile([HL, B], F32, tag="tanhc")
            nc.scalar.activation(tct[:], cn[:], AF.Tanh)
            hn = gpool.tile([HL, B], F32, tag="hn")
            nc.vector.tensor_tensor(hn[:], og[:], tct[:], ALU.mult)
            return hn, cn

        def allgather_h(hn, gname, to_sbuf):
            """AllGather [HL, B] slices across cores -> dram [ncores, HL, B];
            optionally reload the full [H, B] into sbuf k-tiles."""
            bounce = dpool.tile([HL, B], F32, tag=f"{gname}_in")
            nc.gpsimd.dma_start(bounce[:], hn[:])
            gout = dpool.tile([ncores, HL, B], F32, tag=f"{gname}_out")
            nc.gpsimd.collective_compute(
                "AllGather", ALU.bypass,
                replica_groups=[list(range(ncores))],
                ins=[bounce[:].opt()], outs=[gout[:].opt()])
            if not to_sbuf:
                return None, gout
            hall = kpool.tile([128, len(kH), B], F32, tag=f"{gname}_all")
            gflat = gout[:].rearrange("c h b -> (c h) b")
            for i, (o, sz) in enumerate(kH):
                nc.scalar.dma_start(_r(hall[:sz, i, :]), _r(gflat[o:o + sz, :]))
            return hall, gout

        # layer 0: inputs = x_aug (K=D) + h0[0] (K=H)
        h0T0_s = kpool.tile([128, len(kH), B], F32)
        for i, (o, sz) in enumerate(kH):
            nc.scalar.dma_start(_r(h0T0_s[:sz, i, :]), _r(h0T0[o:o + sz, :]))
        h0n, c0n = lstm_layer(
            W0T, b0_s,
            hprevT_src=lambda i: (h0T0_s[:kH[i][1], i, :], kH[i][1]),
            c0T_src=c0T0, nk_x=len(kD),
            xsrc=lambda i: (xa_s[:kD[i][1], i, :], kD[i][1]))
        nc.gpsimd.dma_start(hh0_o[:], h0n[:])
        nc.gpsimd.dma_start(hc0_o[:], c0n[:])
        h0n_all, _ = allgather_h(h0n, "g0", to_sbuf=True)

        # layer 1: inputs = h0_new (K=H) + h0[1] (K=H)
        h0T1_s = kpool.tile([128, len(kH), B], F32)
        for i, (o, sz) in enumerate(kH):
            nc.scalar.dma_start(_r(h0T1_s[:sz, i, :]), _r(h0T1[o:o + sz, :]))
        h1n, c1n = lstm_layer(
            W1T, b1_s,
            hprevT_src=lambda i: (h0T1_s[:kH[i][1], i, :], kH[i][1]),
            c0T_src=c0T1, nk_x=len(kH),
            xsrc=lambda i: (h0n_all[:kH[i][1], i, :], kH[i][1]))
        nc.gpsimd.dma_start(hh1_o[:], h1n[:])
        nc.gpsimd.dma_start(hc1_o[:], c1n[:])
        _, g1out = allgather_h(h1n, "g1", to_sbuf=False)

        # ---------- phase D: heads, on this core's own batch slice ----------
        # Slice the gathered h1_new down to our BL batch columns using the
        # runtime core id (the gather is laid out [core, HL, B]).
        rid = nc.gpsimd.partition_id()
        h1loc = kpool.tile([128, len(kH), BL], F32)
        g1v = g1out[:].rearrange("c h (r b) -> (c h) r b", b=BL)
        for i, (o, sz) in enumerate(kH):
            nc.gpsimd.dma_start(_r(h1loc[:sz, i, :]),
                                _r(g1v[o:o + sz, bass.ds(rid, 1), :]))

        def head(mcols, func, bias_ap, out_sb):
            co, csz = mcols
            ps = pp.tile([128, BL], F32, tag="ps")
            for i, (o, sz) in enumerate(kH):
                wt = wpool.tile([128, 128], F32, tag="wh")
                nc.scalar.dma_start(_r(wt[:sz, :csz]), _r(WheadT[o:o + sz, co:co + csz]))
                nc.tensor.matmul(ps[:csz, :BL], _r(wt[:sz, :csz]),
                                 _r(h1loc[:sz, i, :]),
                                 start=(i == 0), stop=(i == len(kH) - 1))
            nc.scalar.activation(out_sb, ps[:csz, :BL], func, bias=bias_ap)

        vt_s = spool.tile([M, BL], F32, tag="vt")
        for mo, msz in mM:
            head((mo, msz), AF.Tanh, bh_s[mo:mo + msz, 0:1], vt_s[mo:mo + msz, :])
        d_s = spool.tile([1, BL], F32, tag="d")
        head((M, 1), AF.Sigmoid, bh_s[0:1, 1:2], d_s[:])
        for mi, (mo, msz) in enumerate(mD):
            o_s = spool.tile([128, BL], F32, tag="ot")
            head((M + 1 + mo, msz), AF.Tanh, bh_s[:msz, 2 + mi:3 + mi],
                 o_s[:msz, :])
            nc.gpsimd.dma_start(otT_o[mo:mo + msz, :], o_s[:msz, :])

        # vt also goes (untransposed) into the last Val row
        vtp = pp.tile([BL, 128], F32, tag="ps")
        nc.tensor.transpose(vtp[:, :M], vt_s[:], ident_s[:M, :M])
        vtn = spool.tile([BL, M], F32, tag="vtn")
        nc.vector.tensor_copy(vtn[:], vtp[:, :M])
        nc.gpsimd.dma_start(val_o[T1:T1 + 1, :, :], vtn[:])
        nc.gpsimd.dma_start(stg_o[T1:T1 + 1, :], d_s[:])

        # ---------- phase E: stg clamp, suffix sums, coef ----------
        stgc = stgpool.tile([128, NTT, BL], F32)
        for t in range(NTT):
            raw = spool.tile([128, BL], F32, tag="praw")
            nc.scalar.dma_start(raw[:], pstg[128 * t:128 * (t + 1), :])
            nc.vector.tensor_scalar_min(stgc[:, t, :], raw[:], 0.0)
            nc.scalar.dma_start(stg_o[128 * t:128 * (t + 1), :], stgc[:, t, :])

        # per-tile totals: ones_col.T @ stgc  -> [1, NTT*BL] in one matmul
        tsp = pp.tile([1, NTT * BL], F32, tag="ps")
        nc.tensor.matmul(tsp[:], ones_s[:, 0:1],
                         stgc[:].rearrange("p t b -> p (t b)"))
        tsrow = spool.tile([1, (NTT + 1) * BL], F32, tag="tsrow")
        nc.vector.tensor_copy(tsrow[:, :NTT * BL], tsp[:])
        nc.vector.tensor_copy(tsrow[:, NTT * BL:], d_s[:])
        # partition-scatter via DRAM bounce: [1,(NTT+1)*BL] -> [(NTT+1), BL]
        tb = dpool.tile([NTT + 1, BL], F32, tag="tsb")
        nc.gpsimd.dma_start(tb[:].rearrange("t b -> (t b)")[None, :], tsrow[:])
        ts17 = spool.tile([NTT + 1, BL], F32, tag="ts17")
        nc.gpsimd.dma_start(ts17[:], tb[:])
        # exclusive tail per tile (includes dt): triT.T @ ts17 -> [NTT, BL]
        tailp = pp.tile([NTT, BL], F32, tag="ps")
        nc.tensor.matmul(tailp[:], triT_s[:], ts17[:])
        tail_sb = spool.tile([NTT, BL], F32, tag="tailsb")
        nc.vector.tensor_copy(tail_sb[:], tailp[:])
        tb2 = dpool.tile([NTT, BL], F32, tag="tsb2")
        nc.gpsimd.dma_start(tb2[:], tail_sb[:])
        tailrow = spool.tile([1, NTT * BL], F32, tag="tailrow")
        nc.gpsimd.dma_start(tailrow[:],
                            tb2[:].rearrange("t b -> (t b)")[None, :])

        coef = stgpool.tile([128, NTT, BL], F32)
        for t in range(NTT):
            sf = pp.tile([128, BL], F32, tag="ps")
            nc.tensor.matmul(sf[:], tri128_s[:], stgc[:, t, :],
                             start=True, stop=False)
            nc.tensor.matmul(sf[:], ones_s[0:1, :],
                             tailrow[:, t * BL:(t + 1) * BL],
                             start=False, stop=True)
            tmp = spool.tile([128, BL], F32, tag="ctmp")
            nc.vector.tensor_tensor(tmp[:], stgc[:, t, :], sf[:], ALU.subtract)
            nc.vector.tensor_scalar_add(tmp[:], tmp[:], 1.0)
            nc.vector.tensor_scalar_min(tmp[:], tmp[:], 0.0)
            nc.vector.tensor_tensor(coef[:, t, :], stgc[:, t, :], tmp[:], ALU.min)

        # ---------- phase F: stream prev_Val: copy out + reduce into rt ----------
        # Per tile: DMA in, DMA copy out, scale in place by coef (broadcast
        # over m), then ones-vector reduction matmuls (f32r, N=512) over the
        # partition (t) axis, accumulated into an sbuf row rt[(b m)].
        MAC_F32R = True
        BF16 = mybir.dt.bfloat16
        ones_bf = cpool.tile([128, 1], BF16)
        nc.vector.tensor_copy(ones_bf[:], ones_s[:, 0:1])
        rt_row = spool.tile([1, BL * M], F32, tag="rtrow")
        nc.vector.memset(rt_row[:], 0.0)
        pvv = pval[:].rearrange("t b m -> t (b m)")
        vvv = val_o[:].rearrange("t b m -> t (b m)")
        FW = BL * M
        CH = 512
        nch = (FW + CH - 1) // CH
        bpool = es.enter_context(
            tc.tile_pool(name="scaled", bufs=2 if MAC_F32R else 3))
        for t in range(NTT):
            vt_t = vpool.tile([128, FW], F32, tag="val")
            nc.sync.dma_start(vt_t[:], pvv[128 * t:128 * (t + 1), :])
            nc.sync.dma_start(vvv[128 * t:128 * (t + 1), :], vt_t[:])
            cb = coef[:, t, :, None].broadcast_to([128, BL, M])
            if MAC_F32R:
                # DVE-produced f32r scaled copy (verifier requires an
                # f32r-producing instruction, which DMA loads are not)
                sc_t = bpool.tile([128, FW], F32, tag="sc", name="sc")
                nc.vector.tensor_tensor(
                    _r(sc_t[:].rearrange("p (b m) -> p b m", m=M)),
                    vt_t[:].rearrange("p (b m) -> p b m", m=M), cb, ALU.mult)
            else:
                sc_t = bpool.tile([128, FW], BF16, tag="sc")
                nc.vector.tensor_tensor(
                    sc_t[:].rearrange("p (b m) -> p b m", m=M),
                    vt_t[:].rearrange("p (b m) -> p b m", m=M), cb, ALU.mult)
            for c in range(nch):
                co, csz = c * CH, min(CH, FW - c * CH)
                rp = pp.tile([1, CH], F32, tag="ps", name="rp")
                lhs1 = _r(ones_r[:]) if MAC_F32R else ones_bf[:]
                rhs1 = _r(sc_t[:, co:co + csz]) if MAC_F32R else sc_t[:, co:co + csz]
                nc.tensor.matmul(rp[:, :csz], lhs1, rhs1)
                nc.vector.tensor_tensor(rt_row[:, co:co + csz],
                                        rt_row[:, co:co + csz],
                                        rp[:, :csz], ALU.add)
        nc.gpsimd.dma_start(rt_o[:].rearrange("b m -> (b m)")[None, :],
                            rt_row[:])

    nc.compile()
    return nc


_PROG_CACHE = {}


def _get_program(key, **kw):
    if key not in _PROG_CACHE:
        _PROG_CACHE[key] = build_program(**kw)
    return _PROG_CACHE[key]


def make_in_maps(inputs, T1, B, M, D, H, ncores):
    """Host-side data prep: transposes/slices only (no real computation)."""
    f = np.float32
    g = {k: np.asarray(v, dtype=f) for k, v in inputs.items()}
    BL, HL = B // ncores, H // ncores
    NTT = T1 // 128
    DG = 4

    xprT = np.ascontiguousarray(np.concatenate([g["x"][0], g["prev_read"]], 1).T)
    WprojT = np.ascontiguousarray(g["Wproj"].T)
    W0Tfull = np.concatenate([g["Wih0"], g["Whh0"]], axis=1).T  # [D+H, 4H]
    W1Tfull = np.concatenate([g["Wih1"], g["Whh1"]], axis=1).T  # [2H, 4H]
    b0full = g["bih0"] + g["bhh0"]
    b1full = g["bih1"] + g["bhh1"]
    h0T0 = np.ascontiguousarray(g["h0"][0].T)
    h0T1 = np.ascontiguousarray(g["h0"][1].T)
    WheadT = np.ascontiguousarray(
        np.concatenate([g["Wv"], g["Wd"], g["Wo"]], axis=0).T)  # [H, M+1+D]
    nmD = (D + 127) // 128
    bhead = np.zeros((128, 2 + nmD), f)
    bhead[:M, 0] = g["bv"]
    bhead[0, 1] = g["bd"][0]
    for mi in range(nmD):
        sz = min(128, D - mi * 128)
        bhead[:sz, 2 + mi] = g["bo"][mi * 128:mi * 128 + sz]

    tri128 = np.tril(np.ones((128, 128), f))           # tri128[j,i]=1 iff j>=i
    triT = np.zeros((NTT + 1, NTT), f)                 # triT[j,k]=1 iff j>k
    for k in range(NTT):
        triT[k + 1:, k] = 1.0
    allones = np.ones((128, 128), f)
    ident = np.eye(128, dtype=f)

    in_maps = []
    for r in range(ncores):
        hs = slice(r * HL, (r + 1) * HL)
        bs = slice(r * BL, (r + 1) * BL)
        gate_cols = np.concatenate(
            [np.arange(gg * H + r * HL, gg * H + (r + 1) * HL)
             for gg in range(DG)])
        in_maps.append({
            "xprT": xprT,
            "WprojT": WprojT,
            "W0T": np.ascontiguousarray(W0Tfull[:, gate_cols]),
            "b0": np.ascontiguousarray(
                b0full[gate_cols].reshape(DG, HL).T),
            "W1T": np.ascontiguousarray(W1Tfull[:, gate_cols]),
            "b1": np.ascontiguousarray(
                b1full[gate_cols].reshape(DG, HL).T),
            "h0T0": h0T0, "h0T1": h0T1,
            "c0T0": np.ascontiguousarray(g["c0"][0].T[hs]),
            "c0T1": np.ascontiguousarray(g["c0"][1].T[hs]),
            "WheadT": WheadT, "bhead": bhead,
            "pstg": np.ascontiguousarray(g["prev_stg"][:, bs]),
            "pval": np.ascontiguousarray(g["prev_Val"][:, bs]),
            "tri128": tri128, "triT": triT,
            "allones": allones, "ident": ident,
        })
    return in_maps


def assemble_outputs(results, T1, B, M, D, H, ncores):
    f = np.float32
    BL, HL = B // ncores, H // ncores
    ot = np.empty((1, B, D), f)
    Val = np.empty((T1 + 1, B, M), f)
    stg = np.empty((T1 + 1, B), f)
    hh = np.empty((2, B, H), f)
    hc = np.empty((2, B, H), f)
    rt = np.empty((B, M), f)
    for r in range(ncores):
        hs = slice(r * HL, (r + 1) * HL)
        bs = slice(r * BL, (r + 1) * BL)
        res = results[r]
        ot[0, bs, :] = res["otT"].T
        Val[:, bs, :] = res["val"]
        stg[:, bs] = res["stg"]
        hh[0, :, hs] = res["hh0"].T
        hh[1, :, hs] = res["hh1"].T
        hc[0, :, hs] = res["hc0"].T
        hc[1, :, hs] = res["hc1"].T
        rt[bs, :] = res["rt"]
    return (ot, Val, stg, hh, hc, rt)


def kernel(**inputs):
    nc = _get_program("full", T1=T1, B=B, M=M, D=D, H=H, ncores=NCORES)
    in_maps = make_in_maps(inputs, T1, B, M, D, H, NCORES)
    res = run_bass_kernel_spmd(nc, in_maps, list(range(NCORES)))
    return assemble_outputs(res.results, T1, B, M, D, H, NCORES)


# revision 22
# speedup vs baseline: 1.2574x; 1.0553x over previous
"""Trainium2 Bass kernel for the nn_Controller problem.

Math background (verified against the reference scans):
  - The push scan collapses: since ut = sigmoid(...) >= 0, the carry u stays
    >= 0 forever, so s_i = min(prev_stg_i, 0) and stg = [min(prev_stg,0), dt].
  - The read scan is a suffix sum: read_i = 1 - sum_{j>i} stg_j, so
    coef_i = min(stg_i, min(1 - S_{i+1}, 0)) and rt = sum_i coef_i * Val_i.
    (coef at the freshly-pushed slot is min(dt,0) = 0, so vt never enters rt.)

Distribution over 8 cores:
  - LSTM/projection/head phase: feature-parallel over H (each core computes a
    H/8 slice of the hidden state for the full batch, weights sliced per core);
    two AllGathers (h0_new, h1_new) between layers.
  - Stack phase (the memory-heavy part): data-parallel over batch B. Each core
    streams its [T1, B/8, M] slice of prev_Val once: every tile is copied to
    the Val output and reduced against coef into rt on the PE.

Layout convention on device: feature-major ("transposed") [feat, batch] so
that batch is the matmul moving dimension and gate biases are per-partition.
The host wrapper prepares transposed inputs and re-transposes tiny outputs.
"""

from contextlib import ExitStack

import numpy as np

import concourse.bass as bass
import concourse.bacc as bacc
import concourse.mybir as mybir
import concourse.tile as tile
from concourse.bass_utils import run_bass_kernel_spmd

F32 = mybir.dt.float32
F32R = mybir.dt.float32r
AF = mybir.ActivationFunctionType
ALU = mybir.AluOpType


def _r(ap):
    """View an fp32 AP as float32r for fast PE matmuls (1 cyc/row at N>=256)."""
    return ap.bitcast(F32R)

# Full-size problem constants
B, T1, M, D, H, L = 256, 2048, 128, 512, 1024, 2
NCORES = 8


def _chunks(total, step=128):
    return [(o, min(step, total - o)) for o in range(0, total, step)]


def build_program(T1, B, M, D, H, ncores, val_bufs=4):
    """Build the SPMD bass program (same program on every core)."""
    assert B % ncores == 0 and H % ncores == 0
    assert T1 % 128 == 0 and D % 128 == 0 and H % 128 == 0 and M <= 128
    BL = B // ncores          # per-core batch slice for the stack phase
    HL = H // ncores          # per-core feature slice of the hidden state
    NTT = T1 // 128           # number of t-tiles in the stack phase
    DG = 4                    # gates per LSTM cell

    nc = bacc.Bacc("TRN2", target_bir_lowering=False, debug=False,
                   enable_asserts=False, num_devices=ncores)

    def din(name, shape):
        return nc.dram_tensor(name, list(shape), F32, kind="ExternalInput")

    def dout(name, shape):
        return nc.dram_tensor(name, list(shape), F32, kind="ExternalOutput")

    # ---- inputs (per-core arrays supplied via in_maps) ----
    xprT = din("xprT", (D + M, B))          # concat(x, prev_read).T, replicated
    WprojT = din("WprojT", (D + M, D))      # Wproj.T, replicated
    W0T = din("W0T", (D + H, DG * HL))      # per-core gate-column slice
    b0 = din("b0", (HL, DG))
    W1T = din("W1T", (2 * H, DG * HL))
    b1 = din("b1", (HL, DG))
    h0T0 = din("h0T0", (H, B))              # h0[0].T, replicated
    h0T1 = din("h0T1", (H, B))
    c0T0 = din("c0T0", (HL, B))             # c0[0].T feature slice
    c0T1 = din("c0T1", (HL, B))
    WheadT = din("WheadT", (H, M + 1 + D))  # [Wv; Wd; Wo].T, replicated
    nmD = (D + 127) // 128
    bhead = din("bhead", (128, 2 + nmD))
    pstg = din("pstg", (T1, BL))            # prev_stg batch slice
    pval = din("pval", (T1, BL, M))         # prev_Val batch slice
    tri128 = din("tri128", (128, 128))      # tri128[j,i] = 1 if j >= i
    triT = din("triT", (NTT + 1, NTT))      # triT[j,k] = 1 if j > k
    allones = din("allones", (128, 128))
    ident = din("ident", (128, 128))

    # ---- outputs (per-core) ----
    otT_o = dout("otT", (D, BL))
    val_o = dout("val", (T1 + 1, BL, M))
    stg_o = dout("stg", (T1 + 1, BL))
    hh0_o = dout("hh0", (HL, B))
    hh1_o = dout("hh1", (HL, B))
    hc0_o = dout("hc0", (HL, B))
    hc1_o = dout("hc1", (HL, B))
    rt_o = dout("rt", (BL, M))

    kD = _chunks(D)           # k-tiles over D
    kDM = _chunks(D + M)      # k-tiles over D+M (xpr)
    kH = _chunks(H)           # k-tiles over H
    mD = _chunks(D)           # m-chunks of the projection / ot outputs
    mM = _chunks(M)           # m-chunks of vt

    with tile.TileContext(nc) as tc, ExitStack() as es:
        cpool = es.enter_context(tc.tile_pool(name="consts", bufs=1))
        kpool = es.enter_context(tc.tile_pool(name="acts", bufs=1))
        wpool = es.enter_context(tc.tile_pool(name="wstream", bufs=6))
        gpool = es.enter_context(tc.tile_pool(name="gates", bufs=1))
        spool = es.enter_context(tc.tile_pool(name="small", bufs=2))
        stgpool = es.enter_context(tc.tile_pool(name="stg", bufs=1))
        vpool = es.enter_context(tc.tile_pool(name="val", bufs=val_bufs))
        # PSUM budget (8 banks): pp 2 + gates 4
        pp = es.enter_context(tc.tile_pool(name="ps", bufs=2, space="PSUM"))
        ppg = es.enter_context(tc.tile_pool(name="psg", bufs=1, space="PSUM"))
        dpool = es.enter_context(tc.tile_pool(name="dram", bufs=1, space="DRAM"))

        # ---------- constants ----------
        tri128_s = cpool.tile([128, 128], F32)
        nc.scalar.dma_start(tri128_s[:], tri128[:])
        triT_s = cpool.tile([NTT + 1, NTT], F32)
        nc.scalar.dma_start(triT_s[:], triT[:])
        ones_s = cpool.tile([128, 128], F32)
        nc.scalar.dma_start(ones_s[:], allones[:])
        ident_s = cpool.tile([128, 128], F32)
        nc.scalar.dma_start(ident_s[:], ident[:])
        ones_r = cpool.tile([128, 1], F32)
        nc.scalar.dma_start(_r(ones_r[:]), _r(allones[:, 0:1]))
        b0_s = cpool.tile([HL, DG], F32)
        nc.scalar.dma_start(b0_s[:], b0[:])
        b1_s = cpool.tile([HL, DG], F32)
        nc.scalar.dma_start(b1_s[:], b1[:])
        bh_s = cpool.tile([128, 2 + nmD], F32)
        nc.scalar.dma_start(bh_s[:], bhead[:])

        # ---------- phase A: x_aug.T = Wproj @ concat(x, prev_read).T ----------
        xpr_s = kpool.tile([128, len(kDM), B], F32)
        for i, (o, sz) in enumerate(kDM):
            nc.scalar.dma_start(_r(xpr_s[:sz, i, :]), _r(xprT[o:o + sz, :]))
        xa_s = kpool.tile([128, len(mD), B], F32)
        for mi, (mo, msz) in enumerate(mD):
            ps = pp.tile([128, B], F32, tag="ps")
            for i, (o, sz) in enumerate(kDM):
                wt = wpool.tile([128, max(128, DG * HL)], F32, tag="w")
                nc.scalar.dma_start(_r(wt[:sz, :msz]), _r(WprojT[o:o + sz, mo:mo + msz]))
                nc.tensor.matmul(ps[:msz, :B], _r(wt[:sz, :msz]),
                                 _r(xpr_s[:sz, i, :]),
                                 start=(i == 0), stop=(i == len(kDM) - 1))
            nc.vector.tensor_copy(_r(xa_s[:msz, mi, :]), ps[:msz, :B])

        # ---------- LSTM layers ----------
        def lstm_layer(WT, b_s, hprevT_src, c0T_src, nk_x, xsrc):
            """One feature-sliced LSTM cell. Returns (h_new, c_new) [HL, B].
            xsrc(i)/hprevT_src(i) -> (AP [sz, B], sz) for the k-tiles of the
            cell input and the recurrent input. WT rows are 128-strided."""
            gps = [ppg.tile([HL, B], F32, tag=f"g{g}", name=f"g{g}")
                   for g in range(DG)]
            nk = nk_x + len(kH)
            for i in range(nk):
                src, sz = xsrc(i) if i < nk_x else hprevT_src(i - nk_x)
                wt = wpool.tile([128, max(128, DG * HL)], F32, tag="w")
                weng = nc.scalar if i % 2 else nc.sync
                weng.dma_start(_r(wt[:sz, :DG * HL]),
                               _r(WT[128 * i:128 * i + sz, :]))
                for g in range(DG):
                    nc.tensor.matmul(gps[g][:, :],
                                     _r(wt[:sz, g * HL:(g + 1) * HL]),
                                     _r(src), start=(i == 0), stop=(i == nk - 1))
            ig = gpool.tile([HL, B], F32, tag="ig")
            fg = gpool.tile([HL, B], F32, tag="fg")
            gg = gpool.tile([HL, B], F32, tag="gg")
            og = gpool.tile([HL, B], F32, tag="og")
            nc.scalar.activation(ig[:], gps[0][:, :], AF.Sigmoid, bias=b_s[:, 0:1])
            nc.scalar.activation(fg[:], gps[1][:, :], AF.Sigmoid, bias=b_s[:, 1:2])
            nc.scalar.activation(gg[:], gps[2][:, :], AF.Tanh, bias=b_s[:, 2:3])
            nc.scalar.activation(og[:], gps[3][:, :], AF.Sigmoid, bias=b_s[:, 3:4])
            c0s = gpool.tile([HL, B], F32, tag="c0l")
            nc.scalar.dma_start(c0s[:], c0T_src[:])
            cn = gpool.tile([HL, B], F32, tag="cn")
            nc.vector.tensor_tensor(cn[:], fg[:], c0s[:], ALU.mult)
            nc.vector.tensor_tensor(ig[:], ig[:], gg[:], ALU.mult)
            nc.vector.tensor_tensor(cn[:], cn[:], ig[:], ALU.add)
            tct = gpool.tile([HL, B], F32, tag="tanhc")
            nc.scalar.activation(tct[:], cn[:], AF.Tanh)
            hn = gpool.tile([HL, B], F32, tag="hn")
            nc.vector.tensor_tensor(hn[:], og[:], tct[:], ALU.mult)
            return hn, cn

        def allgather_h(hn, gname, to_sbuf):
            """AllGather [HL, B] slices across cores -> dram [ncores, HL, B];
            optionally reload the full [H, B] into sbuf k-tiles."""
            bounce = dpool.tile([HL, B], F32, tag=f"{gname}_in")
            nc.gpsimd.dma_start(bounce[:], hn[:])
            gout = dpool.tile([ncores, HL, B], F32, tag=f"{gname}_out")
            nc.gpsimd.collective_compute(
                "AllGather", ALU.bypass,
                replica_groups=[list(range(ncores))],
                ins=[bounce[:].opt()], outs=[gout[:].opt()])
            if not to_sbuf:
                return None, gout
            hall = kpool.tile([128, len(kH), B], F32, tag=f"{gname}_all")
            gflat = gout[:].rearrange("c h b -> (c h) b")
            for i, (o, sz) in enumerate(kH):
                (nc.scalar if i % 2 else nc.sync).dma_start(
                    _r(hall[:sz, i, :]), _r(gflat[o:o + sz, :]))
            return hall, gout

        # layer 0: inputs = x_aug (K=D) + h0[0] (K=H)
        h0T0_s = kpool.tile([128, len(kH), B], F32)
        for i, (o, sz) in enumerate(kH):
            (nc.scalar if i % 2 else nc.sync).dma_start(
                _r(h0T0_s[:sz, i, :]), _r(h0T0[o:o + sz, :]))
        h0n, c0n = lstm_layer(
            W0T, b0_s,
            hprevT_src=lambda i: (h0T0_s[:kH[i][1], i, :], kH[i][1]),
            c0T_src=c0T0, nk_x=len(kD),
            xsrc=lambda i: (xa_s[:kD[i][1], i, :], kD[i][1]))
        nc.gpsimd.dma_start(hh0_o[:], h0n[:])
        nc.gpsimd.dma_start(hc0_o[:], c0n[:])
        h0n_all, _ = allgather_h(h0n, "g0", to_sbuf=True)

        # layer 1: inputs = h0_new (K=H) + h0[1] (K=H)
        h0T1_s = kpool.tile([128, len(kH), B], F32)
        for i, (o, sz) in enumerate(kH):
            (nc.scalar if i % 2 else nc.sync).dma_start(
                _r(h0T1_s[:sz, i, :]), _r(h0T1[o:o + sz, :]))
        h1n, c1n = lstm_layer(
            W1T, b1_s,
            hprevT_src=lambda i: (h0T1_s[:kH[i][1], i, :], kH[i][1]),
            c0T_src=c0T1, nk_x=len(kH),
            xsrc=lambda i: (h0n_all[:kH[i][1], i, :], kH[i][1]))
        nc.gpsimd.dma_start(hh1_o[:], h1n[:])
        nc.gpsimd.dma_start(hc1_o[:], c1n[:])
        _, g1out = allgather_h(h1n, "g1", to_sbuf=False)

        # ---------- phase D: heads, on this core's own batch slice ----------
        # Slice the gathered h1_new down to our BL batch columns using the
        # runtime core id (the gather is laid out [core, HL, B]).
        rid = nc.gpsimd.partition_id()
        h1loc = kpool.tile([128, len(kH), BL], F32)
        g1v = g1out[:].rearrange("c h (r b) -> (c h) r b", b=BL)
        for i, (o, sz) in enumerate(kH):
            nc.gpsimd.dma_start(_r(h1loc[:sz, i, :]),
                                _r(g1v[o:o + sz, bass.ds(rid, 1), :]))

        def head(mcols, func, bias_ap, out_sb):
            co, csz = mcols
            ps = pp.tile([128, BL], F32, tag="ps")
            for i, (o, sz) in enumerate(kH):
                wt = wpool.tile([128, 128], F32, tag="wh", bufs=4)
                weng = nc.scalar if i % 2 else nc.sync
                weng.dma_start(_r(wt[:sz, :csz]),
                               _r(WheadT[o:o + sz, co:co + csz]))
                nc.tensor.matmul(ps[:csz, :BL], _r(wt[:sz, :csz]),
                                 _r(h1loc[:sz, i, :]),
                                 start=(i == 0), stop=(i == len(kH) - 1))
            nc.scalar.activation(out_sb, ps[:csz, :BL], func, bias=bias_ap)

        vt_s = spool.tile([M, BL], F32, tag="vt")
        for mo, msz in mM:
            head((mo, msz), AF.Tanh, bh_s[mo:mo + msz, 0:1], vt_s[mo:mo + msz, :])
        d_s = spool.tile([1, BL], F32, tag="d")
        head((M, 1), AF.Sigmoid, bh_s[0:1, 1:2], d_s[:])
        for mi, (mo, msz) in enumerate(mD):
            o_s = spool.tile([128, BL], F32, tag="ot")
            head((M + 1 + mo, msz), AF.Tanh, bh_s[:msz, 2 + mi:3 + mi],
                 o_s[:msz, :])
            nc.gpsimd.dma_start(otT_o[mo:mo + msz, :], o_s[:msz, :])

        # vt also goes (untransposed) into the last Val row
        vtp = pp.tile([BL, 128], F32, tag="ps")
        nc.tensor.transpose(vtp[:, :M], vt_s[:], ident_s[:M, :M])
        vtn = spool.tile([BL, M], F32, tag="vtn")
        nc.vector.tensor_copy(vtn[:], vtp[:, :M])
        nc.gpsimd.dma_start(val_o[T1:T1 + 1, :, :], vtn[:])
        nc.gpsimd.dma_start(stg_o[T1:T1 + 1, :], d_s[:])

        # ---------- phase E: stg clamp, suffix sums, coef ----------
        stgc = stgpool.tile([128, NTT, BL], F32)
        for t in range(NTT):
            raw = spool.tile([128, BL], F32, tag="praw")
            nc.scalar.dma_start(raw[:], pstg[128 * t:128 * (t + 1), :])
            nc.vector.tensor_scalar_min(stgc[:, t, :], raw[:], 0.0)
            nc.scalar.dma_start(stg_o[128 * t:128 * (t + 1), :], stgc[:, t, :])

        # per-tile totals: ones_col.T @ stgc  -> [1, NTT*BL] in one matmul
        tsp = pp.tile([1, NTT * BL], F32, tag="ps")
        nc.tensor.matmul(tsp[:], ones_s[:, 0:1],
                         stgc[:].rearrange("p t b -> p (t b)"))
        tsrow = spool.tile([1, NTT * BL], F32, tag="tsrow")
        nc.vector.tensor_copy(tsrow[:], tsp[:])
        # exclusive suffix over tile totals (no dt yet): serial DVE chain
        tail0 = spool.tile([1, NTT * BL], F32, tag="tail0")
        nc.vector.memset(tail0[:, (NTT - 1) * BL:], 0.0)
        for k in range(NTT - 2, -1, -1):
            nc.vector.tensor_tensor(tail0[:, k * BL:(k + 1) * BL],
                                    tail0[:, (k + 1) * BL:(k + 2) * BL],
                                    tsrow[:, (k + 1) * BL:(k + 2) * BL], ALU.add)
        # within-tile inclusive suffix (independent of dt): tri128.T @ stgc
        sfx = stgpool.tile([128, NTT, BL], F32)
        for t in range(NTT):
            sf = pp.tile([128, BL], F32, tag="ps")
            nc.tensor.matmul(sf[:], tri128_s[:], stgc[:, t, :])
            nc.vector.tensor_copy(sfx[:, t, :], sf[:])
        # tail including dt, then coef = min(stg, min(1 - (S_excl + tail), 0))
        taild = spool.tile([1, NTT * BL], F32, tag="taild")
        nc.vector.tensor_tensor(
            taild[:].rearrange("o (t b) -> o t b", b=BL),
            tail0[:].rearrange("o (t b) -> o t b", b=BL),
            d_s[:, None, :].broadcast_to([1, NTT, BL]), ALU.add)
        coef = stgpool.tile([128, NTT, BL], F32)
        for t in range(NTT):
            tb = pp.tile([128, BL], F32, tag="ps", name="tb")
            nc.tensor.matmul(tb[:], ones_s[0:1, :],
                             taild[:, t * BL:(t + 1) * BL])
            tmp = spool.tile([128, BL], F32, tag="ctmp")
            nc.vector.tensor_tensor(tmp[:], stgc[:, t, :], sfx[:, t, :],
                                    ALU.subtract)
            nc.vector.tensor_tensor(tmp[:], tmp[:], tb[:], ALU.subtract)
            nc.vector.tensor_scalar_add(tmp[:], tmp[:], 1.0)
            nc.vector.tensor_scalar_min(tmp[:], tmp[:], 0.0)
            nc.vector.tensor_tensor(coef[:, t, :], stgc[:, t, :], tmp[:], ALU.min)

        # ---------- phase F: stream prev_Val: copy out + reduce into rt ----------
        # Per tile: DMA in, DMA copy out, scale in place by coef (broadcast
        # over m), then ones-vector reduction matmuls (f32r, N=512) over the
        # partition (t) axis, accumulated into an sbuf row rt[(b m)].
        MAC_F32R = True
        BF16 = mybir.dt.bfloat16
        ones_bf = cpool.tile([128, 1], BF16)
        nc.vector.tensor_copy(ones_bf[:], ones_s[:, 0:1])
        rt_row = spool.tile([1, BL * M], F32, tag="rtrow")
        nc.vector.memset(rt_row[:], 0.0)
        pvv = pval[:].rearrange("t b m -> t (b m)")
        vvv = val_o[:].rearrange("t b m -> t (b m)")
        FW = BL * M
        CH = 512
        nch = (FW + CH - 1) // CH
        bpool = es.enter_context(
            tc.tile_pool(name="scaled", bufs=2 if MAC_F32R else 3))
        for t in range(NTT):
            vt_t = vpool.tile([128, FW], F32, tag="val")
            nc.sync.dma_start(vt_t[:], pvv[128 * t:128 * (t + 1), :])
            nc.sync.dma_start(vvv[128 * t:128 * (t + 1), :], vt_t[:])
            cb = coef[:, t, :, None].broadcast_to([128, BL, M])
            if MAC_F32R:
                # DVE-produced f32r scaled copy (verifier requires an
                # f32r-producing instruction, which DMA loads are not)
                sc_t = bpool.tile([128, FW], F32, tag="sc", name="sc")
                nc.vector.tensor_tensor(
                    _r(sc_t[:].rearrange("p (b m) -> p b m", m=M)),
                    vt_t[:].rearrange("p (b m) -> p b m", m=M), cb, ALU.mult)
            else:
                sc_t = bpool.tile([128, FW], BF16, tag="sc")
                nc.vector.tensor_tensor(
                    sc_t[:].rearrange("p (b m) -> p b m", m=M),
                    vt_t[:].rearrange("p (b m) -> p b m", m=M), cb, ALU.mult)
            for c in range(nch):
                co, csz = c * CH, min(CH, FW - c * CH)
                rp = pp.tile([1, CH], F32, tag="ps", name="rp")
                lhs1 = _r(ones_r[:]) if MAC_F32R else ones_bf[:]
                rhs1 = _r(sc_t[:, co:co + csz]) if MAC_F32R else sc_t[:, co:co + csz]
                nc.tensor.matmul(rp[:, :csz], lhs1, rhs1)
                nc.vector.tensor_tensor(rt_row[:, co:co + csz],
                                        rt_row[:, co:co + csz],
                                        rp[:, :csz], ALU.add)
        nc.gpsimd.dma_start(rt_o[:].rearrange("b m -> (b m)")[None, :],
                            rt_row[:])

    nc.compile()
    return nc


_PROG_CACHE = {}


def _get_program(key, **kw):
    if key not in _PROG_CACHE:
        _PROG_CACHE[key] = build_program(**kw)
    return _PROG_CACHE[key]


def make_in_maps(inputs, T1, B, M, D, H, ncores):
    """Host-side data prep: transposes/slices only (no real computation)."""
    f = np.float32
    g = {k: np.asarray(v, dtype=f) for k, v in inputs.items()}
    BL, HL = B // ncores, H // ncores
    NTT = T1 // 128
    DG = 4

    xprT = np.ascontiguousarray(np.concatenate([g["x"][0], g["prev_read"]], 1).T)
    WprojT = np.ascontiguousarray(g["Wproj"].T)
    W0Tfull = np.concatenate([g["Wih0"], g["Whh0"]], axis=1).T  # [D+H, 4H]
    W1Tfull = np.concatenate([g["Wih1"], g["Whh1"]], axis=1).T  # [2H, 4H]
    b0full = g["bih0"] + g["bhh0"]
    b1full = g["bih1"] + g["bhh1"]
    h0T0 = np.ascontiguousarray(g["h0"][0].T)
    h0T1 = np.ascontiguousarray(g["h0"][1].T)
    WheadT = np.ascontiguousarray(
        np.concatenate([g["Wv"], g["Wd"], g["Wo"]], axis=0).T)  # [H, M+1+D]
    nmD = (D + 127) // 128
    bhead = np.zeros((128, 2 + nmD), f)
    bhead[:M, 0] = g["bv"]
    bhead[0, 1] = g["bd"][0]
    for mi in range(nmD):
        sz = min(128, D - mi * 128)
        bhead[:sz, 2 + mi] = g["bo"][mi * 128:mi * 128 + sz]

    tri128 = np.tril(np.ones((128, 128), f))           # tri128[j,i]=1 iff j>=i
    triT = np.zeros((NTT + 1, NTT), f)                 # triT[j,k]=1 iff j>k
    for k in range(NTT):
        triT[k + 1:, k] = 1.0
    allones = np.ones((128, 128), f)
    ident = np.eye(128, dtype=f)

    in_maps = []
    for r in range(ncores):
        hs = slice(r * HL, (r + 1) * HL)
        bs = slice(r * BL, (r + 1) * BL)
        gate_cols = np.concatenate(
            [np.arange(gg * H + r * HL, gg * H + (r + 1) * HL)
             for gg in range(DG)])
        in_maps.append({
            "xprT": xprT,
            "WprojT": WprojT,
            "W0T": np.ascontiguousarray(W0Tfull[:, gate_cols]),
            "b0": np.ascontiguousarray(
                b0full[gate_cols].reshape(DG, HL).T),
            "W1T": np.ascontiguousarray(W1Tfull[:, gate_cols]),
            "b1": np.ascontiguousarray(
                b1full[gate_cols].reshape(DG, HL).T),
            "h0T0": h0T0, "h0T1": h0T1,
            "c0T0": np.ascontiguousarray(g["c0"][0].T[hs]),
            "c0T1": np.ascontiguousarray(g["c0"][1].T[hs]),
            "WheadT": WheadT, "bhead": bhead,
            "pstg": np.ascontiguousarray(g["prev_stg"][:, bs]),
            "pval": np.ascontiguousarray(g["prev_Val"][:, bs]),
            "tri128": tri128, "triT": triT,
            "allones": allones, "ident": ident,
        })
    return in_maps


def assemble_outputs(results, T1, B, M, D, H, ncores):
    f = np.float32
    BL, HL = B // ncores, H // ncores
    ot = np.empty((1, B, D), f)
    Val = np.empty((T1 + 1, B, M), f)
    stg = np.empty((T1 + 1, B), f)
    hh = np.empty((2, B, H), f)
    hc = np.empty((2, B, H), f)
    rt = np.empty((B, M), f)
    for r in range(ncores):
        hs = slice(r * HL, (r + 1) * HL)
        bs = slice(r * BL, (r + 1) * BL)
        res = results[r]
        ot[0, bs, :] = res["otT"].T
        Val[:, bs, :] = res["val"]
        stg[:, bs] = res["stg"]
        hh[0, :, hs] = res["hh0"].T
        hh[1, :, hs] = res["hh1"].T
        hc[0, :, hs] = res["hc0"].T
        hc[1, :, hs] = res["hc1"].T
        rt[bs, :] = res["rt"]
    return (ot, Val, stg, hh, hc, rt)


def kernel(**inputs):
    nc = _get_program("full", T1=T1, B=B, M=M, D=D, H=H, ncores=NCORES)
    in_maps = make_in_maps(inputs, T1, B, M, D, H, NCORES)
    res = run_bass_kernel_spmd(nc, in_maps, list(range(NCORES)))
    return assemble_outputs(res.results, T1, B, M, D, H, NCORES)


# revision 23
# speedup vs baseline: 1.3484x; 1.0724x over previous
"""Trainium2 Bass kernel for the nn_Controller problem.

Math background (verified against the reference scans):
  - The push scan collapses: since ut = sigmoid(...) >= 0, the carry u stays
    >= 0 forever, so s_i = min(prev_stg_i, 0) and stg = [min(prev_stg,0), dt].
  - The read scan is a suffix sum: read_i = 1 - sum_{j>i} stg_j, so
    coef_i = min(stg_i, min(1 - S_{i+1}, 0)) and rt = sum_i coef_i * Val_i.
    (coef at the freshly-pushed slot is min(dt,0) = 0, so vt never enters rt.)

Distribution over 8 cores:
  - LSTM/projection/head phase: feature-parallel over H (each core computes a
    H/8 slice of the hidden state for the full batch, weights sliced per core);
    two AllGathers (h0_new, h1_new) between layers.
  - Stack phase (the memory-heavy part): data-parallel over batch B. Each core
    streams its [T1, B/8, M] slice of prev_Val once: every tile is copied to
    the Val output and reduced against coef into rt on the PE.

Layout convention on device: feature-major ("transposed") [feat, batch] so
that batch is the matmul moving dimension and gate biases are per-partition.
The host wrapper prepares transposed inputs and re-transposes tiny outputs.
"""

from contextlib import ExitStack

import numpy as np

import concourse.bass as bass
import concourse.bacc as bacc
import concourse.mybir as mybir
import concourse.tile as tile
from concourse.bass_utils import run_bass_kernel_spmd

F32 = mybir.dt.float32
F32R = mybir.dt.float32r
AF = mybir.ActivationFunctionType
ALU = mybir.AluOpType


def _r(ap):
    """View an fp32 AP as float32r for fast PE matmuls (1 cyc/row at N>=256)."""
    return ap.bitcast(F32R)

# Full-size problem constants
B, T1, M, D, H, L = 256, 2048, 128, 512, 1024, 2
NCORES = 8


def _chunks(total, step=128):
    return [(o, min(step, total - o)) for o in range(0, total, step)]


def build_program(T1, B, M, D, H, ncores, val_bufs=4):
    """Build the SPMD bass program (same program on every core)."""
    assert B % ncores == 0 and H % ncores == 0
    assert T1 % 128 == 0 and D % 128 == 0 and H % 128 == 0 and M <= 128
    BL = B // ncores          # per-core batch slice for the stack phase
    HL = H // ncores          # per-core feature slice of the hidden state
    NTT = T1 // 128           # number of t-tiles in the stack phase
    DG = 4                    # gates per LSTM cell

    nc = bacc.Bacc("TRN2", target_bir_lowering=False, debug=False,
                   enable_asserts=False, num_devices=ncores)

    def din(name, shape):
        return nc.dram_tensor(name, list(shape), F32, kind="ExternalInput")

    def dout(name, shape):
        return nc.dram_tensor(name, list(shape), F32, kind="ExternalOutput")

    # ---- inputs (per-core arrays supplied via in_maps) ----
    xprT = din("xprT", (D + M, B))          # concat(x, prev_read).T, replicated
    WprojT = din("WprojT", (D + M, D))      # Wproj.T, replicated
    W0T = din("W0T", (D + H, DG * HL))      # per-core gate-column slice
    b0 = din("b0", (HL, DG))
    W1T = din("W1T", (2 * H, DG * HL))
    b1 = din("b1", (HL, DG))
    h0T0 = din("h0T0", (H, B))              # h0[0].T, replicated
    h0T1 = din("h0T1", (H, B))
    c0T0 = din("c0T0", (HL, B))             # c0[0].T feature slice
    c0T1 = din("c0T1", (HL, B))
    WheadT = din("WheadT", (H, M + 1 + D))  # [Wv; Wd; Wo].T, replicated
    nmD = (D + 127) // 128
    bhead = din("bhead", (128, 2 + nmD))
    pstg = din("pstg", (T1, BL))            # prev_stg batch slice
    pval = din("pval", (T1, BL, M))         # prev_Val batch slice
    tri128 = din("tri128", (128, 128))      # tri128[j,i] = 1 if j >= i
    triT = din("triT", (NTT + 1, NTT))      # triT[j,k] = 1 if j > k
    allones = din("allones", (128, 128))
    ident = din("ident", (128, 128))

    # ---- outputs (per-core) ----
    otT_o = dout("otT", (D, BL))
    val_o = dout("val", (T1 + 1, BL, M))
    stg_o = dout("stg", (T1 + 1, BL))
    hh0_o = dout("hh0", (HL, B))
    hh1_o = dout("hh1", (HL, B))
    hc0_o = dout("hc0", (HL, B))
    hc1_o = dout("hc1", (HL, B))
    rt_o = dout("rt", (BL, M))

    kD = _chunks(D)           # k-tiles over D
    kDM = _chunks(D + M)      # k-tiles over D+M (xpr)
    kH = _chunks(H)           # k-tiles over H
    mD = _chunks(D)           # m-chunks of the projection / ot outputs
    mM = _chunks(M)           # m-chunks of vt

    with tile.TileContext(nc) as tc, ExitStack() as es:
        cpool = es.enter_context(tc.tile_pool(name="consts", bufs=1))
        kpool = es.enter_context(tc.tile_pool(name="acts", bufs=1))
        wpool = es.enter_context(tc.tile_pool(name="wstream", bufs=6))
        gpool = es.enter_context(tc.tile_pool(name="gates", bufs=1))
        spool = es.enter_context(tc.tile_pool(name="small", bufs=2))
        stgpool = es.enter_context(tc.tile_pool(name="stg", bufs=1))
        vpool = es.enter_context(tc.tile_pool(name="val", bufs=val_bufs))
        # PSUM budget (8 banks): pp 2 + gates 4
        pp = es.enter_context(tc.tile_pool(name="ps", bufs=2, space="PSUM"))
        ppg = es.enter_context(tc.tile_pool(name="psg", bufs=1, space="PSUM"))
        dpool = es.enter_context(tc.tile_pool(name="dram", bufs=1, space="DRAM"))

        # ---------- constants ----------
        tri128_s = cpool.tile([128, 128], F32)
        nc.scalar.dma_start(tri128_s[:], tri128[:])
        triT_s = cpool.tile([NTT + 1, NTT], F32)
        nc.scalar.dma_start(triT_s[:], triT[:])
        ones_s = cpool.tile([128, 128], F32)
        nc.scalar.dma_start(ones_s[:], allones[:])
        ident_s = cpool.tile([128, 128], F32)
        nc.scalar.dma_start(ident_s[:], ident[:])
        ones_r = cpool.tile([128, 1], F32)
        nc.scalar.dma_start(_r(ones_r[:]), _r(allones[:, 0:1]))
        b0_s = cpool.tile([HL, DG], F32)
        nc.scalar.dma_start(b0_s[:], b0[:])
        b1_s = cpool.tile([HL, DG], F32)
        nc.scalar.dma_start(b1_s[:], b1[:])
        bh_s = cpool.tile([128, 2 + nmD], F32)
        nc.scalar.dma_start(bh_s[:], bhead[:])

        # ---------- phase A: x_aug.T = Wproj @ concat(x, prev_read).T ----------
        xpr_s = kpool.tile([128, len(kDM), B], F32)
        for i, (o, sz) in enumerate(kDM):
            nc.scalar.dma_start(_r(xpr_s[:sz, i, :]), _r(xprT[o:o + sz, :]))
        xa_s = kpool.tile([128, len(mD), B], F32)
        for mi, (mo, msz) in enumerate(mD):
            ps = pp.tile([128, B], F32, tag="ps")
            for i, (o, sz) in enumerate(kDM):
                wt = wpool.tile([128, max(128, DG * HL)], F32, tag="w")
                nc.scalar.dma_start(_r(wt[:sz, :msz]), _r(WprojT[o:o + sz, mo:mo + msz]))
                nc.tensor.matmul(ps[:msz, :B], _r(wt[:sz, :msz]),
                                 _r(xpr_s[:sz, i, :]),
                                 start=(i == 0), stop=(i == len(kDM) - 1))
            nc.vector.tensor_copy(_r(xa_s[:msz, mi, :]), ps[:msz, :B])

        # ---------- LSTM layers ----------
        def lstm_layer(WT, b_s, hprevT_src, c0T_src, nk_x, xsrc):
            """One feature-sliced LSTM cell. Returns (h_new, c_new) [HL, B].
            xsrc(i)/hprevT_src(i) -> (AP [sz, B], sz) for the k-tiles of the
            cell input and the recurrent input. WT rows are 128-strided."""
            gps = [ppg.tile([HL, B], F32, tag=f"g{g}", name=f"g{g}")
                   for g in range(DG)]
            nk = nk_x + len(kH)
            for i in range(nk):
                src, sz = xsrc(i) if i < nk_x else hprevT_src(i - nk_x)
                wt = wpool.tile([128, max(128, DG * HL)], F32, tag="w")
                weng = nc.scalar if i % 2 else nc.sync
                weng.dma_start(_r(wt[:sz, :DG * HL]),
                               _r(WT[128 * i:128 * i + sz, :]))
                for g in range(DG):
                    nc.tensor.matmul(gps[g][:, :],
                                     _r(wt[:sz, g * HL:(g + 1) * HL]),
                                     _r(src), start=(i == 0), stop=(i == nk - 1))
            ig = gpool.tile([HL, B], F32, tag="ig")
            fg = gpool.tile([HL, B], F32, tag="fg")
            gg = gpool.tile([HL, B], F32, tag="gg")
            og = gpool.tile([HL, B], F32, tag="og")
            nc.scalar.activation(ig[:], gps[0][:, :], AF.Sigmoid, bias=b_s[:, 0:1])
            nc.scalar.activation(fg[:], gps[1][:, :], AF.Sigmoid, bias=b_s[:, 1:2])
            nc.scalar.activation(gg[:], gps[2][:, :], AF.Tanh, bias=b_s[:, 2:3])
            nc.scalar.activation(og[:], gps[3][:, :], AF.Sigmoid, bias=b_s[:, 3:4])
            c0s = gpool.tile([HL, B], F32, tag="c0l")
            nc.scalar.dma_start(c0s[:], c0T_src[:])
            cn = gpool.tile([HL, B], F32, tag="cn")
            nc.vector.tensor_tensor(cn[:], fg[:], c0s[:], ALU.mult)
            nc.vector.tensor_tensor(ig[:], ig[:], gg[:], ALU.mult)
            nc.vector.tensor_tensor(cn[:], cn[:], ig[:], ALU.add)
            tct = gpool.tile([HL, B], F32, tag="tanhc")
            nc.scalar.activation(tct[:], cn[:], AF.Tanh)
            hn = gpool.tile([HL, B], F32, tag="hn")
            nc.vector.tensor_tensor(hn[:], og[:], tct[:], ALU.mult)
            return hn, cn

        def allgather_h(hn, gname, to_sbuf):
            """AllGather [HL, B] slices across cores -> dram [ncores, HL, B];
            optionally reload the full [H, B] into sbuf k-tiles."""
            bounce = dpool.tile([HL, B], F32, tag=f"{gname}_in")
            nc.gpsimd.dma_start(bounce[:], hn[:])
            gout = dpool.tile([ncores, HL, B], F32, tag=f"{gname}_out")
            nc.gpsimd.collective_compute(
                "AllGather", ALU.bypass,
                replica_groups=[list(range(ncores))],
                ins=[bounce[:].opt()], outs=[gout[:].opt()])
            if not to_sbuf:
                return None, gout
            hall = kpool.tile([128, len(kH), B], F32, tag=f"{gname}_all")
            gflat = gout[:].rearrange("c h b -> (c h) b")
            for i, (o, sz) in enumerate(kH):
                (nc.scalar if i % 2 else nc.sync).dma_start(
                    _r(hall[:sz, i, :]), _r(gflat[o:o + sz, :]))
            return hall, gout

        # layer 0: inputs = x_aug (K=D) + h0[0] (K=H)
        h0T0_s = kpool.tile([128, len(kH), B], F32)
        for i, (o, sz) in enumerate(kH):
            (nc.scalar if i % 2 else nc.sync).dma_start(
                _r(h0T0_s[:sz, i, :]), _r(h0T0[o:o + sz, :]))
        h0n, c0n = lstm_layer(
            W0T, b0_s,
            hprevT_src=lambda i: (h0T0_s[:kH[i][1], i, :], kH[i][1]),
            c0T_src=c0T0, nk_x=len(kD),
            xsrc=lambda i: (xa_s[:kD[i][1], i, :], kD[i][1]))
        nc.gpsimd.dma_start(hh0_o[:], h0n[:])
        nc.gpsimd.dma_start(hc0_o[:], c0n[:])
        h0n_all, _ = allgather_h(h0n, "g0", to_sbuf=True)

        # layer 1: inputs = h0_new (K=H) + h0[1] (K=H)
        h0T1_s = kpool.tile([128, len(kH), B], F32)
        for i, (o, sz) in enumerate(kH):
            (nc.scalar if i % 2 else nc.sync).dma_start(
                _r(h0T1_s[:sz, i, :]), _r(h0T1[o:o + sz, :]))
        h1n, c1n = lstm_layer(
            W1T, b1_s,
            hprevT_src=lambda i: (h0T1_s[:kH[i][1], i, :], kH[i][1]),
            c0T_src=c0T1, nk_x=len(kH),
            xsrc=lambda i: (h0n_all[:kH[i][1], i, :], kH[i][1]))
        nc.gpsimd.dma_start(hh1_o[:], h1n[:])
        nc.gpsimd.dma_start(hc1_o[:], c1n[:])
        _, g1out = allgather_h(h1n, "g1", to_sbuf=False)

        # ---------- phase D: heads, on this core's own batch slice ----------
        # Slice the gathered h1_new down to our BL batch columns using the
        # runtime core id (the gather is laid out [core, HL, B]).
        rid = nc.gpsimd.partition_id()
        h1loc = kpool.tile([128, len(kH), BL], F32)
        g1v = g1out[:].rearrange("c h (r b) -> (c h) r b", b=BL)
        for i, (o, sz) in enumerate(kH):
            nc.gpsimd.dma_start(_r(h1loc[:sz, i, :]),
                                _r(g1v[o:o + sz, bass.ds(rid, 1), :]))

        def head(mcols, func, bias_ap, out_sb):
            co, csz = mcols
            ps = pp.tile([128, BL], F32, tag="ps")
            for i, (o, sz) in enumerate(kH):
                wt = wpool.tile([128, 128], F32, tag="wh", bufs=4)
                weng = nc.scalar if i % 2 else nc.sync
                weng.dma_start(_r(wt[:sz, :csz]),
                               _r(WheadT[o:o + sz, co:co + csz]))
                nc.tensor.matmul(ps[:csz, :BL], _r(wt[:sz, :csz]),
                                 _r(h1loc[:sz, i, :]),
                                 start=(i == 0), stop=(i == len(kH) - 1))
            nc.scalar.activation(out_sb, ps[:csz, :BL], func, bias=bias_ap)

        vt_s = spool.tile([M, BL], F32, tag="vt")
        for mo, msz in mM:
            head((mo, msz), AF.Tanh, bh_s[mo:mo + msz, 0:1], vt_s[mo:mo + msz, :])
        d_s = spool.tile([1, BL], F32, tag="d")
        head((M, 1), AF.Sigmoid, bh_s[0:1, 1:2], d_s[:])
        for mi, (mo, msz) in enumerate(mD):
            o_s = spool.tile([128, BL], F32, tag="ot")
            head((M + 1 + mo, msz), AF.Tanh, bh_s[:msz, 2 + mi:3 + mi],
                 o_s[:msz, :])
            nc.gpsimd.dma_start(otT_o[mo:mo + msz, :], o_s[:msz, :])

        # vt also goes (untransposed) into the last Val row
        vtp = pp.tile([BL, 128], F32, tag="ps")
        nc.tensor.transpose(vtp[:, :M], vt_s[:], ident_s[:M, :M])
        vtn = spool.tile([BL, M], F32, tag="vtn")
        nc.vector.tensor_copy(vtn[:], vtp[:, :M])
        nc.gpsimd.dma_start(val_o[T1:T1 + 1, :, :], vtn[:])
        nc.gpsimd.dma_start(stg_o[T1:T1 + 1, :], d_s[:])

        # ---------- phase E: stg clamp, suffix sums, coef ----------
        stgc = stgpool.tile([128, NTT, BL], F32)
        for t in range(NTT):
            raw = spool.tile([128, BL], F32, tag="praw")
            nc.scalar.dma_start(raw[:], pstg[128 * t:128 * (t + 1), :])
            nc.vector.tensor_scalar_min(stgc[:, t, :], raw[:], 0.0)
            nc.scalar.dma_start(stg_o[128 * t:128 * (t + 1), :], stgc[:, t, :])

        # per-tile totals: ones_col.T @ stgc  -> [1, NTT*BL] in one matmul
        tsp = pp.tile([1, NTT * BL], F32, tag="ps")
        nc.tensor.matmul(tsp[:], ones_s[:, 0:1],
                         stgc[:].rearrange("p t b -> p (t b)"))
        tsrow = spool.tile([1, NTT * BL], F32, tag="tsrow")
        nc.vector.tensor_copy(tsrow[:], tsp[:])
        # exclusive suffix over tile totals (no dt yet): serial DVE chain
        tail0 = spool.tile([1, NTT * BL], F32, tag="tail0")
        nc.vector.memset(tail0[:, (NTT - 1) * BL:], 0.0)
        for k in range(NTT - 2, -1, -1):
            nc.vector.tensor_tensor(tail0[:, k * BL:(k + 1) * BL],
                                    tail0[:, (k + 1) * BL:(k + 2) * BL],
                                    tsrow[:, (k + 1) * BL:(k + 2) * BL], ALU.add)
        # within-tile inclusive suffix (independent of dt): tri128.T @ stgc
        sfx = stgpool.tile([128, NTT, BL], F32)
        for t in range(NTT):
            sf = pp.tile([128, BL], F32, tag="ps")
            nc.tensor.matmul(sf[:], tri128_s[:], stgc[:, t, :])
            nc.vector.tensor_copy(sfx[:, t, :], sf[:])
        # tail including dt, then coef = min(stg, min(1 - (S_excl + tail), 0))
        taild = spool.tile([1, NTT * BL], F32, tag="taild")
        nc.vector.tensor_tensor(
            taild[:].rearrange("o (t b) -> o t b", b=BL),
            tail0[:].rearrange("o (t b) -> o t b", b=BL),
            d_s[:, None, :].broadcast_to([1, NTT, BL]), ALU.add)
        coef = stgpool.tile([128, NTT, BL], F32)
        for t in range(NTT):
            tb = pp.tile([128, BL], F32, tag="ps", name="tb")
            nc.tensor.matmul(tb[:], ones_s[0:1, :],
                             taild[:, t * BL:(t + 1) * BL])
            tmp = spool.tile([128, BL], F32, tag="ctmp")
            nc.vector.tensor_tensor(tmp[:], stgc[:, t, :], sfx[:, t, :],
                                    ALU.subtract)
            nc.vector.tensor_tensor(tmp[:], tmp[:], tb[:], ALU.subtract)
            nc.vector.tensor_scalar_add(tmp[:], tmp[:], 1.0)
            nc.vector.tensor_scalar_min(tmp[:], tmp[:], 0.0)
            nc.vector.tensor_tensor(coef[:, t, :], stgc[:, t, :], tmp[:], ALU.min)

        # ---------- phase F: stream prev_Val: copy out + reduce into rt ----------
        # Per tile: DMA in, DMA copy out, scale in place by coef (broadcast
        # over m), then ones-vector reduction matmuls (f32r, N=512) over the
        # partition (t) axis, accumulated into an sbuf row rt[(b m)].
        MAC_F32R = True
        BF16 = mybir.dt.bfloat16
        ones_bf = cpool.tile([128, 1], BF16)
        nc.vector.tensor_copy(ones_bf[:], ones_s[:, 0:1])
        rt_row = spool.tile([1, BL * M], F32, tag="rtrow")
        nc.vector.memset(rt_row[:], 0.0)
        pvv = pval[:].rearrange("t b m -> t (b m)")
        vvv = val_o[:].rearrange("t b m -> t (b m)")
        FW = BL * M
        CH = 512
        nch = (FW + CH - 1) // CH
        bpool = es.enter_context(
            tc.tile_pool(name="scaled", bufs=2 if MAC_F32R else 3))
        for t in range(NTT):
            vt_t = vpool.tile([128, FW], F32, tag="val")
            hw = FW // 2
            nc.sync.dma_start(vt_t[:, :hw], pvv[128 * t:128 * (t + 1), :hw])
            nc.sync.dma_start(vt_t[:, hw:], pvv[128 * t:128 * (t + 1), hw:])
            nc.scalar.dma_start(vvv[128 * t:128 * (t + 1), :hw], vt_t[:, :hw])
            nc.scalar.dma_start(vvv[128 * t:128 * (t + 1), hw:], vt_t[:, hw:])
            cb = coef[:, t, :, None].broadcast_to([128, BL, M])
            if MAC_F32R:
                # DVE-produced f32r scaled copy (verifier requires an
                # f32r-producing instruction, which DMA loads are not)
                sc_t = bpool.tile([128, FW], F32, tag="sc", name="sc")
                nc.vector.tensor_tensor(
                    _r(sc_t[:].rearrange("p (b m) -> p b m", m=M)),
                    vt_t[:].rearrange("p (b m) -> p b m", m=M), cb, ALU.mult)
            else:
                sc_t = bpool.tile([128, FW], BF16, tag="sc")
                nc.vector.tensor_tensor(
                    sc_t[:].rearrange("p (b m) -> p b m", m=M),
                    vt_t[:].rearrange("p (b m) -> p b m", m=M), cb, ALU.mult)
            for c in range(nch):
                co, csz = c * CH, min(CH, FW - c * CH)
                rp = pp.tile([1, CH], F32, tag="ps", name="rp")
                lhs1 = _r(ones_r[:]) if MAC_F32R else ones_bf[:]
                rhs1 = _r(sc_t[:, co:co + csz]) if MAC_F32R else sc_t[:, co:co + csz]
                nc.tensor.matmul(rp[:, :csz], lhs1, rhs1)
                nc.vector.tensor_tensor(rt_row[:, co:co + csz],
                                        rt_row[:, co:co + csz],
                                        rp[:, :csz], ALU.add)
        nc.gpsimd.dma_start(rt_o[:].rearrange("b m -> (b m)")[None, :],
                            rt_row[:])

    nc.compile()
    return nc


_PROG_CACHE = {}


def _get_program(key, **kw):
    if key not in _PROG_CACHE:
        _PROG_CACHE[key] = build_program(**kw)
    return _PROG_CACHE[key]


def make_in_maps(inputs, T1, B, M, D, H, ncores):
    """Host-side data prep: transposes/slices only (no real computation)."""
    f = np.float32
    g = {k: np.asarray(v, dtype=f) for k, v in inputs.items()}
    BL, HL = B // ncores, H // ncores
    NTT = T1 // 128
    DG = 4

    xprT = np.ascontiguousarray(np.concatenate([g["x"][0], g["prev_read"]], 1).T)
    WprojT = np.ascontiguousarray(g["Wproj"].T)
    W0Tfull = np.concatenate([g["Wih0"], g["Whh0"]], axis=1).T  # [D+H, 4H]
    W1Tfull = np.concatenate([g["Wih1"], g["Whh1"]], axis=1).T  # [2H, 4H]
    b0full = g["bih0"] + g["bhh0"]
    b1full = g["bih1"] + g["bhh1"]
    h0T0 = np.ascontiguousarray(g["h0"][0].T)
    h0T1 = np.ascontiguousarray(g["h0"][1].T)
    WheadT = np.ascontiguousarray(
        np.concatenate([g["Wv"], g["Wd"], g["Wo"]], axis=0).T)  # [H, M+1+D]
    nmD = (D + 127) // 128
    bhead = np.zeros((128, 2 + nmD), f)
    bhead[:M, 0] = g["bv"]
    bhead[0, 1] = g["bd"][0]
    for mi in range(nmD):
        sz = min(128, D - mi * 128)
        bhead[:sz, 2 + mi] = g["bo"][mi * 128:mi * 128 + sz]

    tri128 = np.tril(np.ones((128, 128), f))           # tri128[j,i]=1 iff j>=i
    triT = np.zeros((NTT + 1, NTT), f)                 # triT[j,k]=1 iff j>k
    for k in range(NTT):
        triT[k + 1:, k] = 1.0
    allones = np.ones((128, 128), f)
    ident = np.eye(128, dtype=f)

    in_maps = []
    for r in range(ncores):
        hs = slice(r * HL, (r + 1) * HL)
        bs = slice(r * BL, (r + 1) * BL)
        gate_cols = np.concatenate(
            [np.arange(gg * H + r * HL, gg * H + (r + 1) * HL)
             for gg in range(DG)])
        in_maps.append({
            "xprT": xprT,
            "WprojT": WprojT,
            "W0T": np.ascontiguousarray(W0Tfull[:, gate_cols]),
            "b0": np.ascontiguousarray(
                b0full[gate_cols].reshape(DG, HL).T),
            "W1T": np.ascontiguousarray(W1Tfull[:, gate_cols]),
            "b1": np.ascontiguousarray(
                b1full[gate_cols].reshape(DG, HL).T),
            "h0T0": h0T0, "h0T1": h0T1,
            "c0T0": np.ascontiguousarray(g["c0"][0].T[hs]),
            "c0T1": np.ascontiguousarray(g["c0"][1].T[hs]),
            "WheadT": WheadT, "bhead": bhead,
            "pstg": np.ascontiguousarray(g["prev_stg"][:, bs]),
            "pval": np.ascontiguousarray(g["prev_Val"][:, bs]),
            "tri128": tri128, "triT": triT,
            "allones": allones, "ident": ident,
        })
    return in_maps


def assemble_outputs(results, T1, B, M, D, H, ncores):
    f = np.float32
    BL, HL = B // ncores, H // ncores
    ot = np.empty((1, B, D), f)
    Val = np.empty((T1 + 1, B, M), f)
    stg = np.empty((T1 + 1, B), f)
    hh = np.empty((2, B, H), f)
    hc = np.empty((2, B, H), f)
    rt = np.empty((B, M), f)
    for r in range(ncores):
        hs = slice(r * HL, (r + 1) * HL)
        bs = slice(r * BL, (r + 1) * BL)
        res = results[r]
        ot[0, bs, :] = res["otT"].T
        Val[:, bs, :] = res["val"]
        stg[:, bs] = res["stg"]
        hh[0, :, hs] = res["hh0"].T
        hh[1, :, hs] = res["hh1"].T
        hc[0, :, hs] = res["hc0"].T
        hc[1, :, hs] = res["hc1"].T
        rt[bs, :] = res["rt"]
    return (ot, Val, stg, hh, hc, rt)


def kernel(**inputs):
    nc = _get_program("full", T1=T1, B=B, M=M, D=D, H=H, ncores=NCORES)
    in_maps = make_in_maps(inputs, T1, B, M, D, H, NCORES)
    res = run_bass_kernel_spmd(nc, in_maps, list(range(NCORES)))
    return assemble_outputs(res.results, T1, B, M, D, H, NCORES)
